# revision 19
# baseline (speedup 1.0000x reference)
"""Trainium2 Bass kernel for CrossEfficientAttention (B=8, C=256, H=W=64, 4 heads).

Sharding: data-parallel over batch B — one sample per NeuronCore, no collectives.

Per-core math (sample x_s, c_s of shape [C, N], N = H*W = 4096):
    Q  = wq @ x_s                      (+ bq, folded into the exp's ACT bias)
    KV = wkv @ c_s                     (bkv[:C] cancels exactly in softmax over N;
                                        bkv[C:] handled as a rank-1 update of W)
    k  = softmax_N(K); q = softmax_head(Q * C**-0.25)
    context = k @ V^T ; out = wo @ (context @ q) + bo

Restructured for the PE array (out = lhsT.T @ rhs, contraction over partitions):
  * KV^T computed directly in [N, C] layout by using c_s tiles as lhsT.
  * k-softmax normalizer: ones-columns appended to V^T give row sums of exp(K)
    in column 256 of the context PSUM accumulator; context rows are then scaled
    by the reciprocal column (per-partition tensor_scalar) — no transposes.
  * wo folded in early: W^T = matmul(lhsT=context, rhs=wo^T) directly in [d, o]
    layout. The per-chunk output is then just out2 = W^T.T @ q.
  * q-softmax denominators via a block-diagonal ones matrix ("Amat",
    blockdiag(J64, J64), identical for both channel halves): Dfull = Amat.T @ eq
    puts D[head(c), n] at every partition c directly — no 4-row D tile, no
    Ln/Exp round-trip, no selector-broadcast matmul.  1/D via the single-
    instruction DVE reciprocal_approx_fast (~51 ULP), and q = eq * rD with the
    multiply split DVE/GpSimd so no single engine becomes the bottleneck.
  * Q phase paced evenly: at iteration j the PE runs Dfull(j-2), out(j-3),
    q(j) back to back; ACT does exp(j) + half of copy(j-3); DVE does
    recip(j-2), half of qmul(j-2), half of copy(j-3); GpSimd does the other
    qmul half.  No long serialized drain — the last chunk's store follows its
    matmuls by ~1.5us, and its two halves ride two DMA queues.

Hard-won scheduling facts (measured on hardware):
  * The PE HAM clock gate passes 4/8 pulses (1.2 GHz) until it has seen a full
    ~3.4us busy window, then 8/8 (2.4 GHz).  The framework preamble takes
    ~6.6us and the first input chunk lands ~2.5us after its trigger, so with
    no warmup the first ~6us of real matmuls run at half clock.  Warmup
    matmuls on a zeroed SBUF tile, sized to end just as the first data lands,
    start the busy window early so real work runs warm almost immediately.
  * DMA trigger instructions cost ~0.65us of issuing-engine queue time each,
    and the first bytes of a queue move ~1.6us after the trigger.  Packet
    count per transfer is rows/16-queues; a 128-col one-c-half slice of cf is
    8 packets/queue and lands ~2us after its trigger.
  * Inputs (x, cp, wkv, wq) and the y output ride in bf16 — halves both
    the input stream and the store stream for ~4.1e-3 rel err; all on-chip
    intermediates stay f32r (same PE speed). fp8 measured 4.6e-2 rel err
    (per-element quantization noise does not average out in the cancelling
    context/out sums), well over the 2e-2 gate.
  * An SBUF->SBUF broadcast DMA (stride-0 source) crashed the device
    (NRT_EXEC_UNIT_UNRECOVERABLE) — broadcasts go through the PE.
"""

import contextlib

import numpy as np

import concourse.bass as bass
import concourse.tile as tile
from concourse import bacc, mybir
from concourse.bass import ts
from concourse.bass_utils import run_bass_kernel_spmd

B, C, H, W = 8, 256, 64, 64
N = H * W
NHEADS = 4
DHEAD = C // NHEADS
NCORES = 8
NSUPER = N // 256          # 16 double-n-tile iterations for the KV phase
NCHUNKS = N // 512         # 8 column chunks for the Q/output phase
SCALE = float(1.0 / np.sqrt(np.sqrt(np.float32(C))))
VW = C + 2                 # V^T tile row width (256 data + 2 ones cols; fp32r needs even free)
WPB = 2 * C + C            # bf16 packed row width per c-half: wkvT|wqT
NWARM = 6                  # warmup matmuls (N=512 cold ~= 0.63us each); sized
                           # to end ~when the first gating DMAs land (~11.5us)
                           # with NO gap, so the HAM busy-window never resets

F32 = mybir.dt.float32
F32R = mybir.dt.float32r
BF16 = mybir.dt.bfloat16
AF = mybir.ActivationFunctionType

_CACHE = {}


def _single_act_table():
    """Scope-patch the activation-table list so the table-load pass resolves
    Exp (and Ln, unused now) to natural_log_exp_and_others (set ids stay
    positional, so only the function lists may change, not the order)."""
    import concourse.bacc as cbacc
    from concourse.hw_specs import get_activation_tables

    @contextlib.contextmanager
    def scope():
        orig = cbacc.get_activation_tables

        def patched(arch):
            tabs = get_activation_tables(arch)
            return {
                k: (v if k == "natural_log_exp_and_others" else set())
                for k, v in tabs.items()
            }

        cbacc.get_activation_tables = patched
        try:
            yield
        finally:
            cbacc.get_activation_tables = orig

    return scope()


def _build(use_bq, use_bo, use_bv, mm_dtype):
    nc = bacc.Bacc("TRN2", target_bir_lowering=False, debug=False)
    # IDT: dtype of the DMA-heavy inputs (x, cp, wkv, wq) — bf16 halves the
    # input stream with one rounding ahead of the softmax averaging.
    # SDT: on-chip intermediates stay f32r.
    IDT = mm_dtype
    SDT = F32R

    x = nc.dram_tensor("x", [C, N], IDT, kind="ExternalInput")
    cp = nc.dram_tensor("cp", [C, N], IDT, kind="ExternalInput")
    wpack = nc.dram_tensor("wpack", [128, 2 * WPB], IDT, kind="ExternalInput")
    wof_d = nc.dram_tensor("wof", [128, 2 * C], F32R, kind="ExternalInput")
    if use_bq:
        bq_s = nc.dram_tensor("bq_s", [C, 1], F32, kind="ExternalInput")
    if use_bo:
        bo_c = nc.dram_tensor("bo_c", [C, 1], F32, kind="ExternalInput")
    if use_bv:
        bv_r = nc.dram_tensor("bv_r", [1, C], F32R, kind="ExternalInput")
        wosum = nc.dram_tensor("wosum", [1, C], F32R, kind="ExternalInput")
    ODT = BF16 if mm_dtype == BF16 else F32
    y = nc.dram_tensor("y", [C, N], ODT, kind="ExternalOutput")

    # DRAM views with the c-half dim split out so one DMA covers both halves
    cp2 = cp[:].rearrange("(u p) n -> p u n", u=2)
    x2 = x[:].rearrange("(u p) n -> p u n", u=2)
    y2 = y[:].rearrange("(u p) n -> p u n", u=2)

    with tile.TileContext(nc) as tc:
        with (
            tc.tile_pool(name="const", bufs=1) as cst,
            tc.tile_pool(name="big", bufs=1) as big,
            tc.tile_pool(name="eqp", bufs=4) as eqp,
            tc.tile_pool(name="rdp", bufs=3) as rdp,
            tc.tile_pool(name="qtp", bufs=3) as qtp,
            tc.tile_pool(name="o2p", bufs=4) as o2p,
        ):
            # --- warmup fodder: zeroed tile, no input dependencies ---
            zt = cst.tile([128, 512], SDT, name="zt")
            nc.vector.memset(zt[:].bitcast(F32), 0.0)

            # --- head-indicator block-diagonal matrix (identical for both
            # c-halves: heads are 64 channels, aligned within each 128-half)
            amat = cst.tile([128, 128], SDT, name="amat")
            nc.vector.memset(amat[:].bitcast(F32), 0.0)
            nc.vector.memset(amat[0:64, 0:64].bitcast(F32), 1.0)
            nc.vector.memset(amat[64:128, 64:128].bitcast(F32), 1.0)

            # manually-rotated V^T ring: ones columns pre-set once
            NVBUF = 4
            v2r = [cst.tile([128, 2 * VW], SDT, name=f"v2_{i}") for i in range(NVBUF)]
            for i in range(NVBUF):
                for h in range(2):
                    o = v2r[i][:, h * VW + C : h * VW + C + 2].bitcast(F32)
                    nc.vector.memset(o, 1.0)

            # --- packed weights; the KV-phase slice (wkvT) rides first ---
            wpk = cst.tile([128, 2 * WPB], IDT, name="wpk")
            wpk3 = wpk[:].rearrange("p (u w) -> p u w", u=2)
            wpack3 = wpack[:].rearrange("p (u w) -> p u w", u=2)
            wof = cst.tile([128, 2 * C], SDT, name="wof")
            wkvT_sb = [wpk[:, u * WPB : u * WPB + 2 * C] for u in range(2)]
            wqT_sb = [wpk[:, u * WPB + 2 * C : u * WPB + 3 * C] for u in range(2)]
            woT_sb = [wof[:, u * C : (u + 1) * C] for u in range(2)]

            cf_sb = big.tile([128, 2, N], IDT, name="cf_sb")
            xf_sb = big.tile([128, 2, N], IDT, name="xf_sb")

            # --- input triggers: early DMA runs at only ~130 GB/s aggregate,
            # so the first-iteration gating set (wkvT both halves + first cf
            # slices) is split ACROSS the two HWDGE queues (sync + scalar) so
            # the halves land in parallel, and the cf gates are 128-col
            # per-c-half slices (32 KB).  Only SP and ACT have HWDGE queues.
            nc.sync.dma_start(out=wpk3[:, 0, 0 : 2 * C], in_=wpack3[:, 0, 0 : 2 * C])
            nc.scalar.dma_start(out=wpk3[:, 1, 0 : 2 * C], in_=wpack3[:, 1, 0 : 2 * C])
            nc.sync.dma_start(out=cf_sb[:, 0:1, 0:128], in_=cp2[:, 0:1, 0:128])
            nc.scalar.dma_start(out=cf_sb[:, 1:2, 0:128], in_=cp2[:, 1:2, 0:128])
            nc.sync.dma_start(out=cf_sb[:, 0:1, 128:256], in_=cp2[:, 0:1, 128:256])
            nc.scalar.dma_start(out=cf_sb[:, 1:2, 128:256], in_=cp2[:, 1:2, 128:256])
            nc.scalar.dma_start(out=cf_sb[:, :, 256:512], in_=cp2[:, :, 256:512])
            # bulk: cf then x, graduated, all on sync (scalar goes back to ACT
            # work; wqT/wof triggers are issued mid-KV-loop below)
            nc.sync.dma_start(out=cf_sb[:, :, 512:1024], in_=cp2[:, :, 512:1024])
            nc.sync.dma_start(out=cf_sb[:, :, 1024:2048], in_=cp2[:, :, 1024:2048])
            nc.sync.dma_start(out=cf_sb[:, :, 2048:4096], in_=cp2[:, :, 2048:4096])
            nc.sync.dma_start(out=xf_sb[:, :, 0:2048], in_=x2[:, :, 0:2048])
            nc.sync.dma_start(out=xf_sb[:, :, 2048:4096], in_=x2[:, :, 2048:4096])
            if use_bq:
                bq_sb = [cst.tile([128, 1], F32, name=f"bq{u}") for u in range(2)]
                for u in range(2):
                    nc.scalar.dma_start(out=bq_sb[u][:], in_=bq_s[ts(u, 128), :])
            if use_bo:
                bo_sb = [cst.tile([128, 1], F32, name=f"bo{u}") for u in range(2)]
                for u in range(2):
                    nc.scalar.dma_start(out=bo_sb[u][:], in_=bo_c[ts(u, 128), :])
            if use_bv:
                bv_sb = cst.tile([1, C], SDT, name="bv_sb")
                nc.scalar.dma_start(out=bv_sb[:], in_=bv_r[:])
                wosum_sb = cst.tile([1, C], SDT, name="wosum_sb")
                nc.scalar.dma_start(out=wosum_sb[:], in_=wosum[:])

            # persistent W^T tiles (filled in the epilogue)
            WT_sb = [cst.tile([128, C], SDT, name=f"WT{u}") for u in range(2)]

            eqs, psDs, qts = {}, {}, {}

            def q_mms_into(j, psQ):
                for t in range(2):
                    for u in range(2):
                        nc.tensor.matmul(
                            psQ[:, t * 512 : (t + 1) * 512],
                            wqT_sb[u][:, ts(t, 128)],
                            xf_sb[:, u, ts(j, 512)],
                            start=(u == 0),
                            stop=(u == 1),
                        )

            def eq_act(j, psQ):
                eq = eqp.tile([128, 1024], SDT, name="eq", tag="eq")
                if use_bq:
                    for t in range(2):
                        nc.scalar.activation(
                            out=eq[:, t * 512 : (t + 1) * 512],
                            in_=psQ[:, t * 512 : (t + 1) * 512],
                            func=AF.Exp,
                            scale=SCALE,
                            bias=bq_sb[t][:],
                        )
                else:
                    nc.scalar.activation(
                        out=eq[:], in_=psQ[:], func=AF.Exp, scale=SCALE
                    )
                eqs[j] = eq

            def dfull_mms(j, pool=None, tag="psD"):
                psD = (pool or psdp).tile([128, 1024], F32, name="psD", tag=tag)
                eq = eqs[j]
                for t in range(2):
                    nc.tensor.matmul(
                        psD[:, t * 512 : (t + 1) * 512],
                        amat[:],
                        eq[:, t * 512 : (t + 1) * 512],
                        start=True,
                        stop=True,
                    )
                psDs[j] = psD

            def recip_qmul(j):
                # GpSimd measured 2x slower than DVE here AND it shares the
                # DVE SBUF port (concurrent GpS+DVE tensor ops ran DVE at
                # 1662ns vs 681 solo) — so the whole multiply rides DVE.
                psD = psDs.pop(j)
                rD = rdp.tile([128, 1024], F32, name="rD", tag="rD")
                nc.vector.reciprocal_approx_fast(out=rD[:], in_=psD[:])
                qt = qtp.tile([128, 1024], SDT, name="qt", tag="qt")
                eq = eqs.pop(j)
                nc.vector.tensor_mul(qt[:], eq[:], rD[:])
                qts[j] = qt

            # ---- warmup: keep the PE (and the HAM busy-window) running from
            # right after the preamble until the first input data lands ----
            pwarm = tc.alloc_tile_pool(name="pswarm", bufs=1, space="PSUM")
            psw = pwarm.tile([128, 512], F32, name="psw")
            for k in range(NWARM):
                nc.tensor.matmul(
                    psw[:], zt[:, 0:128], zt[:],
                    start=True, stop=True, skip_group_check=True,
                )
            pwarm.release()

            # ============ KV phase: context = exp(K) @ [V^T | 1] ============
            # Software-pipelined by one iteration: the PE runs iteration i's
            # KV matmuls and iteration i-1's context matmuls back to back.
            # u is the outer loop so iteration 0's first matmuls gate only on
            # the per-c-half 128-col cf slices.
            pctx = tc.alloc_tile_pool(name="psum_ctx", bufs=1, space="PSUM")
            psCtx = [pctx.tile([128, VW], F32, name=f"psCtx{u}") for u in range(2)]
            pkv = tc.alloc_tile_pool(name="psum_kv", bufs=3, space="PSUM")
            kvsb = tc.alloc_tile_pool(name="kvsb", bufs=3)
            eks = {}

            def kv_mms(i):
                psKV = pkv.tile([128, 1024], F32, name="psKV")
                for u in range(2):
                    for h in range(2):
                        nt = 2 * i + h
                        nc.tensor.matmul(
                            psKV[:, h * 512 : (h + 1) * 512],
                            cf_sb[:, u, ts(nt, 128)],
                            wkvT_sb[u],
                            start=(u == 0),
                            stop=(u == 1),
                            skip_group_check=True,
                        )
                return psKV

            def ctx_mms(i):
                ek = eks.pop(i)
                v2 = v2r[i % NVBUF]
                for h in range(2):
                    for u in range(2):
                        nc.tensor.matmul(
                            psCtx[u][:],
                            ek[:, h, ts(u, 128)],
                            v2[:, h * VW : (h + 1) * VW],
                            start=(i == 0 and h == 0),
                            stop=(i == NSUPER - 1 and h == 1),
                            skip_group_check=True,
                        )

            def kv_post(i, psKV):
                ek = kvsb.tile([128, 2, C], SDT, name="ek")
                nc.scalar.activation(
                    out=ek[:],
                    in_=psKV[:].rearrange("p (h c) -> p h c", h=2)[:, :, 0:C],
                    func=AF.Exp,
                )
                eks[i] = ek
                v2 = v2r[i % NVBUF]
                nc.vector.tensor_copy(
                    v2[:].rearrange("p (h w) -> p h w", h=2)[:, :, 0:C],
                    psKV[:].rearrange("p (h c) -> p h c", h=2)[:, :, C : 2 * C],
                )

            # the first two Q chunks ride inside the KV tail (their PSUM
            # supertiles borrow the KV pool's slots) so their eq tiles are
            # ready the moment the KV phase ends
            for i in range(NSUPER):
                psKV = kv_mms(i)
                if i > 0:
                    ctx_mms(i - 1)
                kv_post(i, psKV)
                if i == 2:
                    # wqT (needed from iter 14) + woT (epilogue): issue these
                    # scalar-queue triggers after the first ek ACTs so they
                    # don't delay the KV pipeline's scalar work
                    nc.scalar.dma_start(
                        out=wpk3[:, :, 2 * C : WPB], in_=wpack3[:, :, 2 * C : WPB]
                    )
                    nc.scalar.dma_start(out=wof[:], in_=wof_d[:])
                if i >= NSUPER - 2:
                    psQ = pkv.tile([128, 1024], F32, name="psKV", tag="psKV")
                    q_mms_into(i - (NSUPER - 2), psQ)
                    eq_act(i - (NSUPER - 2), psQ)
                if i == NSUPER - 1:
                    # chunk 0's whole softmax chain rides the KV tail too
                    # (pkv slots + DVE slack), shortening the post-KV DVE
                    # serial chain — the Q phase is DVE-bound end to end
                    dfull_mms(0, pool=pkv, tag="psKV")
                    recip_qmul(0)
            ctx_mms(NSUPER - 1)
            dfull_mms(1, pool=pkv, tag="psKV")
            recip_qmul(1)
            kvsb.release()
            pkv.release()

            # ===== epilogue part 1 (DVE): normalize context rows =====
            rcol = [cst.tile([128, 1], F32, name=f"rcol{u}") for u in range(2)]
            ctx_sb = [cst.tile([128, C], SDT, name=f"ctx{u}") for u in range(2)]
            for u in range(2):
                nc.vector.reciprocal(rcol[u][:], psCtx[u][:, C : C + 1])
                nc.vector.tensor_scalar_mul(
                    out=ctx_sb[u][:], in0=psCtx[u][:, 0:C], scalar1=rcol[u][:]
                )
            pctx.release()

            # pool stack is LIFO: psq/psd/po/pw pushed after pctx popped.
            # (Program order doesn't serialize engines — the PE still runs
            # dfull(0) right after ctx(15); only data deps matter.)
            psqp = tc.alloc_tile_pool(name="psq", bufs=1, space="PSUM")
            psdp = tc.alloc_tile_pool(name="psd", bufs=1, space="PSUM")
            po = tc.alloc_tile_pool(name="pso", bufs=1, space="PSUM")
            pw = tc.alloc_tile_pool(name="psum_w", bufs=1, space="PSUM")

            # ===== epilogue part 2: fold wo, W^T = ctx.T @ woT =====
            psW = [pw.tile([128, C], F32, name=f"psW{v}") for v in range(2)]
            for v in range(2):
                for u in range(2):
                    nc.tensor.matmul(
                        psW[v][:],
                        ctx_sb[u][:, ts(v, 128)],
                        woT_sb[u],
                        start=(u == 0),
                        stop=(u == 1) and not use_bv,
                        skip_group_check=True,
                    )
                if use_bv:
                    # context gains +bv[d'] per row (sum_n k = 1), so
                    # W^T += bv (X) rowsum(wo): a K=1 rank-1 matmul.
                    nc.tensor.matmul(
                        psW[v][:],
                        bv_sb[:, ts(v, 128)],
                        wosum_sb[:],
                        start=False,
                        stop=True,
                        skip_group_check=True,
                    )
                # WT copy on ACT — the DVE is saturated at the seam with the
                # hoisted recip/qmul chains
                nc.scalar.activation(out=WT_sb[v][:], in_=psW[v][:], func=AF.Copy)

            def out_mms(c, psO):
                qt = qts.pop(c)
                for t in range(2):
                    for u in range(2):
                        nc.tensor.matmul(
                            psO[:, t * 512 : (t + 1) * 512],
                            WT_sb[u][:, ts(t, 128)],
                            qt[:, u * 512 : (u + 1) * 512],
                            start=(u == 0),
                            stop=(u == 1),
                        )

            def out_copy_store(c, psO):
                o2 = o2p.tile([128, 2, 512], ODT, name="o2", tag="o2")
                o2f = o2[:].rearrange("p t n -> p (t n)")
                if use_bo:
                    for t in range(2):
                        nc.vector.tensor_scalar_add(
                            out=o2[:, t, :],
                            in0=psO[:, t * 512 : (t + 1) * 512],
                            scalar1=bo_sb[t][:],
                        )
                    nc.sync.dma_start(out=y2[:, :, ts(c, 512)], in_=o2[:])
                    return
                # o2's flat free layout matches psO's supertile order, so ONE
                # full-width ACT copy does the whole cast (ACT has the slack;
                # DVE is the critical engine in the Q loop).  The last chunk
                # instead splits copy ACT/DVE (DVE is done by then) and its
                # halves ride two DMA queues, so the final store drains fast.
                if c == NCHUNKS - 1:
                    nc.scalar.activation(
                        out=o2[:, 0, :], in_=psO[:, 0:512], func=AF.Copy
                    )
                    nc.sync.dma_start(out=y2[:, 0:1, ts(c, 512)], in_=o2[:, 0:1, :])
                    nc.vector.tensor_copy(o2[:, 1, :], psO[:, 512:1024])
                    nc.scalar.dma_start(out=y2[:, 1:2, ts(c, 512)], in_=o2[:, 1:2, :])
                else:
                    nc.scalar.activation(out=o2f, in_=psO[:], func=AF.Copy)
                    nc.sync.dma_start(out=y2[:, :, ts(c, 512)], in_=o2[:])

            # ===== Q main loop: stages q(j), eq(j), Dfull(j-2), recip/qmul
            # (j-2), out(j-3), copy+store(j-3).  Chunks 0-1's q/eq ran in the
            # KV tail; chunk 0's Dfull/recip/qmul ran in the epilogue. =====
            po2 = None
            psOs = {}
            for j in range(2, NCHUNKS + 2):
                if 2 <= j - 1 <= NCHUNKS - 1:
                    dfull_mms(j - 1)
                if j - 2 >= 0:
                    c = j - 2
                    pool = po if c % 2 == 0 else po2
                    psO = pool.tile([128, 1024], F32, name="psO", tag="psO")
                    out_mms(c, psO)
                    psOs[c] = psO
                if j <= NCHUNKS - 1:
                    psQ = psqp.tile([128, 1024], F32, name="psQ", tag="psQ")
                    q_mms_into(j, psQ)
                    eq_act(j, psQ)
                if 2 <= j - 1 <= NCHUNKS - 1:
                    recip_qmul(j - 1)
                if j - 2 >= 0:
                    out_copy_store(j - 2, psOs.pop(j - 2))
                if j == 2:
                    # pw's banks free after the WT copies -> second out pool
                    pw.release()
                    po2 = tc.alloc_tile_pool(name="pso2", bufs=1, space="PSUM")
            po2.release()
            po.release()
            psdp.release()
            psqp.release()

    nc.compile()
    return nc


def _get_nc(use_bq, use_bo, use_bv, mm_dtype):
    key = (use_bq, use_bo, use_bv, str(mm_dtype))
    if key not in _CACHE:
        with _single_act_table():
            _CACHE[key] = _build(use_bq, use_bo, use_bv, mm_dtype)
    return _CACHE[key]


def _to_mdt(a, mm_dtype):
    if mm_dtype == BF16:
        import ml_dtypes

        return np.ascontiguousarray(a.astype(ml_dtypes.bfloat16))
    return np.ascontiguousarray(a)


def kernel(x, cproj, wq, bq, wkv, bkv, wo, bo, _mm_dtype=BF16, _results_hook=None):
    x = np.ascontiguousarray(np.asarray(x, dtype=np.float32).reshape(B, C, N))
    cf = np.ascontiguousarray(np.asarray(cproj, dtype=np.float32).reshape(B, C, N))
    wq = np.asarray(wq, dtype=np.float32)
    wkv = np.asarray(wkv, dtype=np.float32)
    wo = np.asarray(wo, dtype=np.float32)
    bq = np.asarray(bq, dtype=np.float32)
    bkv = np.asarray(bkv, dtype=np.float32)
    bo = np.asarray(bo, dtype=np.float32)

    use_bq = bool(np.any(bq != 0))
    use_bo = bool(np.any(bo != 0))
    bv = bkv[C:]
    use_bv = bool(np.any(bv != 0))

    wqT = np.ascontiguousarray(wq.T)
    wkvT = np.ascontiguousarray(wkv.T)
    woT = np.ascontiguousarray(wo.T)

    # packed weights: bf16 [wkvT | wqT] per c-half, f32 [woT] per c-half
    wpack = np.zeros((128, 2 * WPB), np.float32)
    wof = np.zeros((128, 2 * C), np.float32)
    for u in range(2):
        r = slice(u * 128, (u + 1) * 128)
        wpack[:, u * WPB : u * WPB + 2 * C] = wkvT[r]
        wpack[:, u * WPB + 2 * C : u * WPB + 3 * C] = wqT[r]
        wof[:, u * C : (u + 1) * C] = woT[r]

    nc = _get_nc(use_bq, use_bo, use_bv, _mm_dtype)

    base = {
        "wpack": _to_mdt(wpack, _mm_dtype),
        "wof": wof,
    }
    if use_bq:
        base["bq_s"] = (SCALE * bq).reshape(C, 1)
    if use_bo:
        base["bo_c"] = bo.reshape(C, 1)
    if use_bv:
        base["bv_r"] = bv.reshape(1, C)
        base["wosum"] = wo.sum(axis=1).reshape(1, C)

    in_maps = [
        dict(base, x=_to_mdt(x[b], _mm_dtype), cp=_to_mdt(cf[b], _mm_dtype))
        for b in range(B)
    ]
    res = run_bass_kernel_spmd(nc, in_maps, list(range(NCORES)))
    if _results_hook is not None:
        _results_hook(res)
    out = np.stack(
        [np.asarray(res.results[b]["y"], dtype=np.float32) for b in range(B)],
        axis=0,
    )
    return out.reshape(B, C, H, W)


# revision 20
# speedup vs baseline: 1.0759x; 1.0759x over previous
"""Trainium2 Bass kernel for CrossEfficientAttention (B=8, C=256, H=W=64, 4 heads).

Sharding: data-parallel over batch B — one sample per NeuronCore, no collectives.

Per-core math (sample x_s, c_s of shape [C, N], N = H*W = 4096):
    Q  = wq @ x_s                      (+ bq, folded into the exp's ACT bias)
    KV = wkv @ c_s                     (bkv[:C] cancels exactly in softmax over N;
                                        bkv[C:] handled as a rank-1 update of W)
    k  = softmax_N(K); q = softmax_head(Q * C**-0.25)
    context = k @ V^T ; out = wo @ (context @ q) + bo

Restructured for the PE array (out = lhsT.T @ rhs, contraction over partitions):
  * KV^T computed directly in [N, C] layout by using c_s tiles as lhsT.
  * k-softmax normalizer: ones-columns appended to V^T give row sums of exp(K)
    in column 256 of the context PSUM accumulator; context rows are then scaled
    by the reciprocal column (per-partition tensor_scalar) — no transposes.
  * wo folded in early: W^T = matmul(lhsT=context, rhs=wo^T) directly in [d, o]
    layout. The per-chunk output is then just out2 = W^T.T @ q.
  * q-softmax denominators via a block-diagonal ones matrix ("Amat",
    blockdiag(J64, J64), identical for both channel halves): Dfull = Amat.T @ eq
    puts D[head(c), n] at every partition c directly — no 4-row D tile, no
    Ln/Exp round-trip, no selector-broadcast matmul.  1/D via the single-
    instruction DVE reciprocal_approx_fast (~51 ULP), and q = eq * rD with the
    multiply split DVE/GpSimd so no single engine becomes the bottleneck.
  * Q phase paced evenly: at iteration j the PE runs Dfull(j-2), out(j-3),
    q(j) back to back; ACT does exp(j) + half of copy(j-3); DVE does
    recip(j-2), half of qmul(j-2), half of copy(j-3); GpSimd does the other
    qmul half.  No long serialized drain — the last chunk's store follows its
    matmuls by ~1.5us, and its two halves ride two DMA queues.

Hard-won scheduling facts (measured on hardware):
  * The PE HAM clock gate passes 4/8 pulses (1.2 GHz) until it has seen a full
    ~3.4us busy window, then 8/8 (2.4 GHz).  The framework preamble takes
    ~6.6us and the first input chunk lands ~2.5us after its trigger, so with
    no warmup the first ~6us of real matmuls run at half clock.  Warmup
    matmuls on a zeroed SBUF tile, sized to end just as the first data lands,
    start the busy window early so real work runs warm almost immediately.
  * DMA trigger instructions cost ~0.65us of issuing-engine queue time each,
    and the first bytes of a queue move ~1.6us after the trigger.  Packet
    count per transfer is rows/16-queues; a 128-col one-c-half slice of cf is
    8 packets/queue and lands ~2us after its trigger.
  * Inputs (x, cp, wkv, wq) and the y output ride in bf16 — halves both
    the input stream and the store stream for ~4.1e-3 rel err; all on-chip
    intermediates stay f32r (same PE speed). fp8 measured 4.6e-2 rel err
    (per-element quantization noise does not average out in the cancelling
    context/out sums), well over the 2e-2 gate.
  * An SBUF->SBUF broadcast DMA (stride-0 source) crashed the device
    (NRT_EXEC_UNIT_UNRECOVERABLE) — broadcasts go through the PE.
"""

import contextlib

import numpy as np

import concourse.bass as bass
import concourse.tile as tile
from concourse import bacc, mybir
from concourse.bass import ts
from concourse.bass_utils import run_bass_kernel_spmd

B, C, H, W = 8, 256, 64, 64
N = H * W
NHEADS = 4
DHEAD = C // NHEADS
NCORES = 8
NSUPER = N // 256          # 16 double-n-tile iterations for the KV phase
NCHUNKS = N // 512         # 8 column chunks for the Q/output phase
SCALE = float(1.0 / np.sqrt(np.sqrt(np.float32(C))))
VW = C + 2                 # V^T tile row width (256 data + 2 ones cols; fp32r needs even free)
WPB = 2 * C + C            # bf16 packed row width per c-half: wkvT|wqT
NWARM = 6                  # warmup matmuls (N=512 cold ~= 0.63us each); sized
                           # to end ~when the first gating DMAs land (~11.5us)
                           # with NO gap, so the HAM busy-window never resets

F32 = mybir.dt.float32
F32R = mybir.dt.float32r
BF16 = mybir.dt.bfloat16
AF = mybir.ActivationFunctionType

_CACHE = {}


def _single_act_table():
    """Scope-patch the activation-table list so the table-load pass resolves
    Exp (and Ln, unused now) to natural_log_exp_and_others (set ids stay
    positional, so only the function lists may change, not the order)."""
    import concourse.bacc as cbacc
    from concourse.hw_specs import get_activation_tables

    @contextlib.contextmanager
    def scope():
        orig = cbacc.get_activation_tables

        def patched(arch):
            tabs = get_activation_tables(arch)
            return {
                k: (v if k == "natural_log_exp_and_others" else set())
                for k, v in tabs.items()
            }

        cbacc.get_activation_tables = patched
        try:
            yield
        finally:
            cbacc.get_activation_tables = orig

    return scope()


def _build(use_bq, use_bo, use_bv, mm_dtype):
    nc = bacc.Bacc("TRN2", target_bir_lowering=False, debug=False)
    # IDT: dtype of the DMA-heavy inputs (x, cp, wkv, wq) — bf16 halves the
    # input stream with one rounding ahead of the softmax averaging.
    # SDT: on-chip intermediates stay f32r.
    IDT = mm_dtype
    SDT = F32R

    x = nc.dram_tensor("x", [C, N], IDT, kind="ExternalInput")
    cp = nc.dram_tensor("cp", [C, N], IDT, kind="ExternalInput")
    wpack = nc.dram_tensor("wpack", [128, 2 * WPB], IDT, kind="ExternalInput")
    wof_d = nc.dram_tensor("wof", [128, 2 * C], F32R, kind="ExternalInput")
    if use_bq:
        bq_s = nc.dram_tensor("bq_s", [C, 1], F32, kind="ExternalInput")
    if use_bo:
        bo_c = nc.dram_tensor("bo_c", [C, 1], F32, kind="ExternalInput")
    if use_bv:
        bv_r = nc.dram_tensor("bv_r", [1, C], F32R, kind="ExternalInput")
        wosum = nc.dram_tensor("wosum", [1, C], F32R, kind="ExternalInput")
    ODT = BF16 if mm_dtype == BF16 else F32
    y = nc.dram_tensor("y", [C, N], ODT, kind="ExternalOutput")

    # DRAM views with the c-half dim split out so one DMA covers both halves
    cp2 = cp[:].rearrange("(u p) n -> p u n", u=2)
    x2 = x[:].rearrange("(u p) n -> p u n", u=2)
    y2 = y[:].rearrange("(u p) n -> p u n", u=2)

    with tile.TileContext(nc) as tc:
        with (
            tc.tile_pool(name="const", bufs=1) as cst,
            tc.tile_pool(name="big", bufs=1) as big,
            tc.tile_pool(name="eqp", bufs=4) as eqp,
            tc.tile_pool(name="rdp", bufs=3) as rdp,
            tc.tile_pool(name="qtp", bufs=3) as qtp,
            tc.tile_pool(name="o2p", bufs=4) as o2p,
        ):
            # --- warmup fodder: zeroed tile, no input dependencies ---
            zt = cst.tile([128, 512], SDT, name="zt")
            nc.vector.memset(zt[:].bitcast(F32), 0.0)

            # --- head-indicator block-diagonal matrix (identical for both
            # c-halves: heads are 64 channels, aligned within each 128-half)
            amat = cst.tile([128, 128], SDT, name="amat")
            nc.vector.memset(amat[:].bitcast(F32), 0.0)
            nc.vector.memset(amat[0:64, 0:64].bitcast(F32), 1.0)
            nc.vector.memset(amat[64:128, 64:128].bitcast(F32), 1.0)

            # manually-rotated V^T ring: ones columns pre-set once
            NVBUF = 4
            v2r = [cst.tile([128, 2 * VW], SDT, name=f"v2_{i}") for i in range(NVBUF)]
            for i in range(NVBUF):
                for h in range(2):
                    o = v2r[i][:, h * VW + C : h * VW + C + 2].bitcast(F32)
                    nc.vector.memset(o, 1.0)

            # --- packed weights; the KV-phase slice (wkvT) rides first ---
            wpk = cst.tile([128, 2 * WPB], IDT, name="wpk")
            wpk3 = wpk[:].rearrange("p (u w) -> p u w", u=2)
            wpack3 = wpack[:].rearrange("p (u w) -> p u w", u=2)
            wof = cst.tile([128, 2 * C], SDT, name="wof")
            wkvT_sb = [wpk[:, u * WPB : u * WPB + 2 * C] for u in range(2)]
            wqT_sb = [wpk[:, u * WPB + 2 * C : u * WPB + 3 * C] for u in range(2)]
            woT_sb = [wof[:, u * C : (u + 1) * C] for u in range(2)]

            cf_sb = big.tile([128, 2, N], IDT, name="cf_sb")
            xf_sb = big.tile([128, 2, N], IDT, name="xf_sb")

            # --- input triggers: early DMA runs at only ~130 GB/s aggregate,
            # so the first-iteration gating set (wkvT both halves + first cf
            # slices) is split ACROSS the two HWDGE queues (sync + scalar) so
            # the halves land in parallel, and the cf gates are 128-col
            # per-c-half slices (32 KB).  Only SP and ACT have HWDGE queues.
            nc.sync.dma_start(out=wpk3[:, 0, 0 : 2 * C], in_=wpack3[:, 0, 0 : 2 * C])
            nc.scalar.dma_start(out=wpk3[:, 1, 0 : 2 * C], in_=wpack3[:, 1, 0 : 2 * C])
            nc.sync.dma_start(out=cf_sb[:, 0:1, 0:128], in_=cp2[:, 0:1, 0:128])
            nc.scalar.dma_start(out=cf_sb[:, 1:2, 0:128], in_=cp2[:, 1:2, 0:128])
            nc.sync.dma_start(out=cf_sb[:, 0:1, 128:256], in_=cp2[:, 0:1, 128:256])
            nc.scalar.dma_start(out=cf_sb[:, 1:2, 128:256], in_=cp2[:, 1:2, 128:256])
            nc.scalar.dma_start(out=cf_sb[:, :, 256:512], in_=cp2[:, :, 256:512])
            # bulk: cf then x, graduated, all on sync (scalar goes back to ACT
            # work; wqT/wof triggers are issued mid-KV-loop below)
            nc.sync.dma_start(out=cf_sb[:, :, 512:1024], in_=cp2[:, :, 512:1024])
            nc.sync.dma_start(out=cf_sb[:, :, 1024:2048], in_=cp2[:, :, 1024:2048])
            nc.sync.dma_start(out=cf_sb[:, :, 2048:4096], in_=cp2[:, :, 2048:4096])
            nc.sync.dma_start(out=xf_sb[:, :, 0:2048], in_=x2[:, :, 0:2048])
            nc.sync.dma_start(out=xf_sb[:, :, 2048:4096], in_=x2[:, :, 2048:4096])
            if use_bq:
                bq_sb = [cst.tile([128, 1], F32, name=f"bq{u}") for u in range(2)]
                for u in range(2):
                    nc.scalar.dma_start(out=bq_sb[u][:], in_=bq_s[ts(u, 128), :])
            if use_bo:
                bo_sb = [cst.tile([128, 1], F32, name=f"bo{u}") for u in range(2)]
                for u in range(2):
                    nc.scalar.dma_start(out=bo_sb[u][:], in_=bo_c[ts(u, 128), :])
            if use_bv:
                bv_sb = cst.tile([1, C], SDT, name="bv_sb")
                nc.scalar.dma_start(out=bv_sb[:], in_=bv_r[:])
                wosum_sb = cst.tile([1, C], SDT, name="wosum_sb")
                nc.scalar.dma_start(out=wosum_sb[:], in_=wosum[:])

            # persistent W^T tiles (filled in the epilogue)
            WT_sb = [cst.tile([128, C], SDT, name=f"WT{u}") for u in range(2)]

            eqs, psDs, qts = {}, {}, {}

            def q_mms_into(j, psQ):
                for t in range(2):
                    for u in range(2):
                        nc.tensor.matmul(
                            psQ[:, t * 512 : (t + 1) * 512],
                            wqT_sb[u][:, ts(t, 128)],
                            xf_sb[:, u, ts(j, 512)],
                            start=(u == 0),
                            stop=(u == 1),
                        )

            def eq_act(j, psQ):
                eq = eqp.tile([128, 1024], SDT, name="eq", tag="eq")
                if use_bq:
                    for t in range(2):
                        nc.scalar.activation(
                            out=eq[:, t * 512 : (t + 1) * 512],
                            in_=psQ[:, t * 512 : (t + 1) * 512],
                            func=AF.Exp,
                            scale=SCALE,
                            bias=bq_sb[t][:],
                        )
                else:
                    nc.scalar.activation(
                        out=eq[:], in_=psQ[:], func=AF.Exp, scale=SCALE
                    )
                eqs[j] = eq

            def dfull_mms(j, pool=None, tag="psD"):
                psD = (pool or psdp).tile([128, 1024], F32, name="psD", tag=tag)
                eq = eqs[j]
                for t in range(2):
                    nc.tensor.matmul(
                        psD[:, t * 512 : (t + 1) * 512],
                        amat[:],
                        eq[:, t * 512 : (t + 1) * 512],
                        start=True,
                        stop=True,
                    )
                psDs[j] = psD

            def recip_qmul(j):
                # GpSimd measured 2x slower than DVE here AND it shares the
                # DVE SBUF port (concurrent GpS+DVE tensor ops ran DVE at
                # 1662ns vs 681 solo) — so the whole multiply rides DVE.
                psD = psDs.pop(j)
                rD = rdp.tile([128, 1024], F32, name="rD", tag="rD")
                nc.vector.reciprocal_approx_fast(out=rD[:], in_=psD[:])
                qt = qtp.tile([128, 1024], SDT, name="qt", tag="qt")
                eq = eqs.pop(j)
                nc.vector.tensor_mul(qt[:], eq[:], rD[:])
                qts[j] = qt

            # ---- warmup: keep the PE (and the HAM busy-window) running from
            # right after the preamble until the first input data lands ----
            pwarm = tc.alloc_tile_pool(name="pswarm", bufs=1, space="PSUM")
            psw = pwarm.tile([128, 512], F32, name="psw")
            for k in range(NWARM):
                nc.tensor.matmul(
                    psw[:], zt[:, 0:128], zt[:],
                    start=True, stop=True, skip_group_check=True,
                )
            pwarm.release()

            # ============ KV phase: context = exp(K) @ [V^T | 1] ============
            # Software-pipelined by one iteration: the PE runs iteration i's
            # KV matmuls and iteration i-1's context matmuls back to back.
            # u is the outer loop so iteration 0's first matmuls gate only on
            # the per-c-half 128-col cf slices.
            pctx = tc.alloc_tile_pool(name="psum_ctx", bufs=1, space="PSUM")
            psCtx = [pctx.tile([128, VW], F32, name=f"psCtx{u}") for u in range(2)]
            pkv = tc.alloc_tile_pool(name="psum_kv", bufs=3, space="PSUM")
            kvsb = tc.alloc_tile_pool(name="kvsb", bufs=3)
            eks = {}

            def kv_mms(i):
                psKV = pkv.tile([128, 1024], F32, name="psKV")
                for u in range(2):
                    for h in range(2):
                        nt = 2 * i + h
                        nc.tensor.matmul(
                            psKV[:, h * 512 : (h + 1) * 512],
                            cf_sb[:, u, ts(nt, 128)],
                            wkvT_sb[u],
                            start=(u == 0),
                            stop=(u == 1),
                            skip_group_check=True,
                        )
                return psKV

            def ctx_mms(i):
                ek = eks.pop(i)
                v2 = v2r[i % NVBUF]
                for h in range(2):
                    for u in range(2):
                        nc.tensor.matmul(
                            psCtx[u][:],
                            ek[:, h, ts(u, 128)],
                            v2[:, h * VW : (h + 1) * VW],
                            start=(i == 0 and h == 0),
                            stop=(i == NSUPER - 1 and h == 1),
                            skip_group_check=True,
                        )

            def kv_post(i, psKV):
                ek = kvsb.tile([128, 2, C], SDT, name="ek")
                nc.scalar.activation(
                    out=ek[:],
                    in_=psKV[:].rearrange("p (h c) -> p h c", h=2)[:, :, 0:C],
                    func=AF.Exp,
                )
                eks[i] = ek
                v2 = v2r[i % NVBUF]
                nc.vector.tensor_copy(
                    v2[:].rearrange("p (h w) -> p h w", h=2)[:, :, 0:C],
                    psKV[:].rearrange("p (h c) -> p h c", h=2)[:, :, C : 2 * C],
                )

            # the first two Q chunks ride inside the KV tail (their PSUM
            # supertiles borrow the KV pool's slots) so their eq tiles are
            # ready the moment the KV phase ends
            for i in range(NSUPER):
                psKV = kv_mms(i)
                if i > 0:
                    ctx_mms(i - 1)
                kv_post(i, psKV)
                if i == 2:
                    # wqT (needed from iter 14) + woT (epilogue): issue these
                    # scalar-queue triggers after the first ek ACTs so they
                    # don't delay the KV pipeline's scalar work
                    nc.scalar.dma_start(
                        out=wpk3[:, :, 2 * C : WPB], in_=wpack3[:, :, 2 * C : WPB]
                    )
                    nc.scalar.dma_start(out=wof[:], in_=wof_d[:])
                if i >= NSUPER - 2:
                    psQ = pkv.tile([128, 1024], F32, name="psKV", tag="psKV")
                    q_mms_into(i - (NSUPER - 2), psQ)
                    eq_act(i - (NSUPER - 2), psQ)
                if i == NSUPER - 1:
                    # chunks 0-1's Dfull matmuls ride the KV tail (pkv
                    # slots), shortening the post-KV DVE serial chain — the
                    # Q phase is DVE-bound end to end.  Their recip/qmuls
                    # are issued AFTER the context normalize: the normalize
                    # feeds W^T (and thereby every out matmul), so it must
                    # lead the DVE queue at the seam.
                    dfull_mms(0, pool=pkv, tag="psKV")
            ctx_mms(NSUPER - 1)
            dfull_mms(1, pool=pkv, tag="psKV")
            kvsb.release()
            pkv.release()

            # ===== epilogue part 1 (DVE): normalize context rows =====
            rcol = [cst.tile([128, 1], F32, name=f"rcol{u}") for u in range(2)]
            ctx_sb = [cst.tile([128, C], SDT, name=f"ctx{u}") for u in range(2)]
            for u in range(2):
                nc.vector.reciprocal(rcol[u][:], psCtx[u][:, C : C + 1])
                nc.vector.tensor_scalar_mul(
                    out=ctx_sb[u][:], in0=psCtx[u][:, 0:C], scalar1=rcol[u][:]
                )
            pctx.release()
            recip_qmul(0)
            recip_qmul(1)

            # pool stack is LIFO: psq/psd/po/pw pushed after pctx popped.
            # (Program order doesn't serialize engines — the PE still runs
            # dfull(0) right after ctx(15); only data deps matter.)
            psqp = tc.alloc_tile_pool(name="psq", bufs=1, space="PSUM")
            psdp = tc.alloc_tile_pool(name="psd", bufs=1, space="PSUM")
            po = tc.alloc_tile_pool(name="pso", bufs=1, space="PSUM")
            pw = tc.alloc_tile_pool(name="psum_w", bufs=1, space="PSUM")

            # ===== epilogue part 2: fold wo, W^T = ctx.T @ woT =====
            psW = [pw.tile([128, C], F32, name=f"psW{v}") for v in range(2)]
            for v in range(2):
                for u in range(2):
                    nc.tensor.matmul(
                        psW[v][:],
                        ctx_sb[u][:, ts(v, 128)],
                        woT_sb[u],
                        start=(u == 0),
                        stop=(u == 1) and not use_bv,
                        skip_group_check=True,
                    )
                if use_bv:
                    # context gains +bv[d'] per row (sum_n k = 1), so
                    # W^T += bv (X) rowsum(wo): a K=1 rank-1 matmul.
                    nc.tensor.matmul(
                        psW[v][:],
                        bv_sb[:, ts(v, 128)],
                        wosum_sb[:],
                        start=False,
                        stop=True,
                        skip_group_check=True,
                    )
                # WT copy on ACT — the DVE is saturated at the seam with the
                # hoisted recip/qmul chains
                nc.scalar.activation(out=WT_sb[v][:], in_=psW[v][:], func=AF.Copy)

            def out_mms(c, psO):
                qt = qts.pop(c)
                for t in range(2):
                    for u in range(2):
                        nc.tensor.matmul(
                            psO[:, t * 512 : (t + 1) * 512],
                            WT_sb[u][:, ts(t, 128)],
                            qt[:, u * 512 : (u + 1) * 512],
                            start=(u == 0),
                            stop=(u == 1),
                        )

            def out_copy_store(c, psO):
                o2 = o2p.tile([128, 2, 512], ODT, name="o2", tag="o2")
                o2f = o2[:].rearrange("p t n -> p (t n)")
                if use_bo:
                    for t in range(2):
                        nc.vector.tensor_scalar_add(
                            out=o2[:, t, :],
                            in0=psO[:, t * 512 : (t + 1) * 512],
                            scalar1=bo_sb[t][:],
                        )
                    nc.sync.dma_start(out=y2[:, :, ts(c, 512)], in_=o2[:])
                    return
                # o2's flat free layout matches psO's supertile order, so ONE
                # full-width ACT copy does the whole cast (ACT has the slack;
                # DVE is the critical engine in the Q loop).  The last chunk
                # instead splits copy ACT/DVE (DVE is done by then) and its
                # halves ride two DMA queues, so the final store drains fast.
                if c == NCHUNKS - 1:
                    nc.scalar.activation(
                        out=o2[:, 0, :], in_=psO[:, 0:512], func=AF.Copy
                    )
                    nc.sync.dma_start(out=y2[:, 0:1, ts(c, 512)], in_=o2[:, 0:1, :])
                    nc.vector.tensor_copy(o2[:, 1, :], psO[:, 512:1024])
                    nc.scalar.dma_start(out=y2[:, 1:2, ts(c, 512)], in_=o2[:, 1:2, :])
                else:
                    nc.scalar.activation(out=o2f, in_=psO[:], func=AF.Copy)
                    nc.sync.dma_start(out=y2[:, :, ts(c, 512)], in_=o2[:])

            # ===== Q main loop: stages q(j), eq(j), Dfull(j-2), recip/qmul
            # (j-2), out(j-3), copy+store(j-3).  Chunks 0-1's q/eq ran in the
            # KV tail; chunk 0's Dfull/recip/qmul ran in the epilogue. =====
            po2 = None
            psOs = {}
            for j in range(2, NCHUNKS + 2):
                if 2 <= j - 1 <= NCHUNKS - 1:
                    dfull_mms(j - 1)
                if j - 2 >= 0:
                    c = j - 2
                    pool = po if c % 2 == 0 else po2
                    psO = pool.tile([128, 1024], F32, name="psO", tag="psO")
                    out_mms(c, psO)
                    psOs[c] = psO
                if j <= NCHUNKS - 1:
                    psQ = psqp.tile([128, 1024], F32, name="psQ", tag="psQ")
                    q_mms_into(j, psQ)
                    eq_act(j, psQ)
                if 2 <= j - 1 <= NCHUNKS - 1:
                    recip_qmul(j - 1)
                if j - 2 >= 0:
                    out_copy_store(j - 2, psOs.pop(j - 2))
                if j == 2:
                    # pw's banks free after the WT copies -> second out pool
                    pw.release()
                    po2 = tc.alloc_tile_pool(name="pso2", bufs=1, space="PSUM")
            po2.release()
            po.release()
            psdp.release()
            psqp.release()

    nc.compile()
    return nc


def _get_nc(use_bq, use_bo, use_bv, mm_dtype):
    key = (use_bq, use_bo, use_bv, str(mm_dtype))
    if key not in _CACHE:
        with _single_act_table():
            _CACHE[key] = _build(use_bq, use_bo, use_bv, mm_dtype)
    return _CACHE[key]


def _to_mdt(a, mm_dtype):
    if mm_dtype == BF16:
        import ml_dtypes

        return np.ascontiguousarray(a.astype(ml_dtypes.bfloat16))
    return np.ascontiguousarray(a)


def kernel(x, cproj, wq, bq, wkv, bkv, wo, bo, _mm_dtype=BF16, _results_hook=None):
    x = np.ascontiguousarray(np.asarray(x, dtype=np.float32).reshape(B, C, N))
    cf = np.ascontiguousarray(np.asarray(cproj, dtype=np.float32).reshape(B, C, N))
    wq = np.asarray(wq, dtype=np.float32)
    wkv = np.asarray(wkv, dtype=np.float32)
    wo = np.asarray(wo, dtype=np.float32)
    bq = np.asarray(bq, dtype=np.float32)
    bkv = np.asarray(bkv, dtype=np.float32)
    bo = np.asarray(bo, dtype=np.float32)

    use_bq = bool(np.any(bq != 0))
    use_bo = bool(np.any(bo != 0))
    bv = bkv[C:]
    use_bv = bool(np.any(bv != 0))

    wqT = np.ascontiguousarray(wq.T)
    wkvT = np.ascontiguousarray(wkv.T)
    woT = np.ascontiguousarray(wo.T)

    # packed weights: bf16 [wkvT | wqT] per c-half, f32 [woT] per c-half
    wpack = np.zeros((128, 2 * WPB), np.float32)
    wof = np.zeros((128, 2 * C), np.float32)
    for u in range(2):
        r = slice(u * 128, (u + 1) * 128)
        wpack[:, u * WPB : u * WPB + 2 * C] = wkvT[r]
        wpack[:, u * WPB + 2 * C : u * WPB + 3 * C] = wqT[r]
        wof[:, u * C : (u + 1) * C] = woT[r]

    nc = _get_nc(use_bq, use_bo, use_bv, _mm_dtype)

    base = {
        "wpack": _to_mdt(wpack, _mm_dtype),
        "wof": wof,
    }
    if use_bq:
        base["bq_s"] = (SCALE * bq).reshape(C, 1)
    if use_bo:
        base["bo_c"] = bo.reshape(C, 1)
    if use_bv:
        base["bv_r"] = bv.reshape(1, C)
        base["wosum"] = wo.sum(axis=1).reshape(1, C)

    in_maps = [
        dict(base, x=_to_mdt(x[b], _mm_dtype), cp=_to_mdt(cf[b], _mm_dtype))
        for b in range(B)
    ]
    res = run_bass_kernel_spmd(nc, in_maps, list(range(NCORES)))
    if _results_hook is not None:
        _results_hook(res)
    out = np.stack(
        [np.asarray(res.results[b]["y"], dtype=np.float32) for b in range(B)],
        axis=0,
    )
    return out.reshape(B, C, H, W)


# revision 30
# speedup vs baseline: 1.1519x; 1.0706x over previous
"""Trainium2 Bass kernel for CrossEfficientAttention (B=8, C=256, H=W=64, 4 heads).

Sharding: data-parallel over batch B — one sample per NeuronCore, no collectives.

Per-core math (sample x_s, c_s of shape [C, N], N = H*W = 4096):
    Q  = wq @ x_s                      (+ bq, folded into the exp's ACT bias)
    KV = wkv @ c_s                     (bkv[:C] cancels exactly in softmax over N;
                                        bkv[C:] handled as a rank-1 update of W)
    k  = softmax_N(K); q = softmax_head(Q * C**-0.25)
    context = k @ V^T ; out = wo @ (context @ q) + bo

Restructured for the PE array (out = lhsT.T @ rhs, contraction over partitions):
  * KV^T computed directly in [N, C] layout by using c_s tiles as lhsT.
  * k-softmax normalizer: ones-columns appended to V^T give row sums of exp(K)
    in column 256 of the context PSUM accumulator; context rows are then scaled
    by the reciprocal column (per-partition tensor_scalar) — no transposes.
  * wo folded in early: W^T = matmul(lhsT=context, rhs=wo^T) directly in [d, o]
    layout. The per-chunk output is then just out2 = W^T.T @ q.
  * q-softmax denominators via a block-diagonal ones matrix ("Amat",
    blockdiag(J64, J64), identical for both channel halves): Dfull = Amat.T @ eq
    puts D[head(c), n] at every partition c directly — no 4-row D tile, no
    Ln/Exp round-trip, no selector-broadcast matmul.  1/D via the single-
    instruction DVE reciprocal_approx_fast (~51 ULP), and q = eq * rD with the
    multiply split DVE/GpSimd so no single engine becomes the bottleneck.
  * Q phase paced evenly: at iteration j the PE runs Dfull(j-2), out(j-3),
    q(j) back to back; ACT does exp(j) + half of copy(j-3); DVE does
    recip(j-2), half of qmul(j-2), half of copy(j-3); GpSimd does the other
    qmul half.  No long serialized drain — the last chunk's store follows its
    matmuls by ~1.5us, and its two halves ride two DMA queues.

Hard-won scheduling facts (measured on hardware):
  * The PE HAM clock gate passes 4/8 pulses (1.2 GHz) until it has seen a full
    ~3.4us busy window, then 8/8 (2.4 GHz).  The framework preamble takes
    ~6.6us and the first input chunk lands ~2.5us after its trigger, so with
    no warmup the first ~6us of real matmuls run at half clock.  Warmup
    matmuls on a zeroed SBUF tile, sized to end just as the first data lands,
    start the busy window early so real work runs warm almost immediately.
  * DMA trigger instructions cost ~0.65us of issuing-engine queue time each,
    and the first bytes of a queue move ~1.6us after the trigger.  Packet
    count per transfer is rows/16-queues; a 128-col one-c-half slice of cf is
    8 packets/queue and lands ~2us after its trigger.
  * Inputs (x, cp, wkv, wq) and the y output ride in bf16 — halves both
    the input stream and the store stream for ~4.1e-3 rel err; all on-chip
    intermediates stay f32r (same PE speed). fp8 measured 4.6e-2 rel err
    (per-element quantization noise does not average out in the cancelling
    context/out sums), well over the 2e-2 gate.
  * An SBUF->SBUF broadcast DMA (stride-0 source) crashed the device
    (NRT_EXEC_UNIT_UNRECOVERABLE) — broadcasts go through the PE.
"""

import contextlib

import numpy as np

import concourse.bass as bass
import concourse.tile as tile
from concourse import bacc, mybir
from concourse.bass import ts
from concourse.bass_utils import run_bass_kernel_spmd

B, C, H, W = 8, 256, 64, 64
N = H * W
NHEADS = 4
DHEAD = C // NHEADS
NCORES = 8
NSUPER = N // 256          # 16 double-n-tile iterations for the KV phase
NCHUNKS = N // 512         # 8 column chunks for the Q/output phase
SCALE = float(1.0 / np.sqrt(np.sqrt(np.float32(C))))
VW = C + 2                 # V^T tile row width (256 data + 2 ones cols; fp32r needs even free)
WPB = 2 * C + C            # bf16 packed row width per c-half: wkvT|wqT
NWARM = 6                  # warmup matmuls (N=512 cold ~= 0.63us each); sized
                           # to end ~when the first gating DMAs land (~11.5us)
                           # with NO gap, so the HAM busy-window never resets

F32 = mybir.dt.float32
F32R = mybir.dt.float32r
BF16 = mybir.dt.bfloat16
AF = mybir.ActivationFunctionType

_CACHE = {}


def _single_act_table():
    """Scope-patch the activation-table list so the table-load pass resolves
    Exp (and Ln, unused now) to natural_log_exp_and_others (set ids stay
    positional, so only the function lists may change, not the order)."""
    import concourse.bacc as cbacc
    from concourse.hw_specs import get_activation_tables

    @contextlib.contextmanager
    def scope():
        orig = cbacc.get_activation_tables

        def patched(arch):
            tabs = get_activation_tables(arch)
            return {
                k: (v if k == "natural_log_exp_and_others" else set())
                for k, v in tabs.items()
            }

        cbacc.get_activation_tables = patched
        try:
            yield
        finally:
            cbacc.get_activation_tables = orig

    return scope()


def _build(use_bq, use_bo, use_bv, mm_dtype):
    nc = bacc.Bacc("TRN2", target_bir_lowering=False, debug=False)
    # IDT: dtype of the DMA-heavy inputs (x, cp, wkv, wq) — bf16 halves the
    # input stream with one rounding ahead of the softmax averaging.
    # SDT: on-chip intermediates stay f32r.
    IDT = mm_dtype
    SDT = F32R
    # KV-context operands (exp(K), V^T) in bf16: the 64 ctx matmuls drop from
    # ~125ns to ~110ns each and the KV phase ends ~1us earlier (everything
    # downstream is serialized behind it).  ~+0.5% rel err via the cancelling
    # context sum — budget is 2e-2, measured ~4e-3 before this.
    KDT = BF16 if mm_dtype == BF16 else F32R

    x = nc.dram_tensor("x", [C, N], IDT, kind="ExternalInput")
    cp = nc.dram_tensor("cp", [C, N], IDT, kind="ExternalInput")
    wpack = nc.dram_tensor("wpack", [128, 2 * WPB], IDT, kind="ExternalInput")
    wof_d = nc.dram_tensor("wof", [128, 2 * C], F32R, kind="ExternalInput")
    if use_bq:
        bq_s = nc.dram_tensor("bq_s", [C, 1], F32, kind="ExternalInput")
    if use_bo:
        bo_c = nc.dram_tensor("bo_c", [C, 1], F32, kind="ExternalInput")
    if use_bv:
        bv_r = nc.dram_tensor("bv_r", [1, C], F32R, kind="ExternalInput")
        wosum = nc.dram_tensor("wosum", [1, C], F32R, kind="ExternalInput")
    ODT = BF16 if mm_dtype == BF16 else F32
    y = nc.dram_tensor("y", [C, N], ODT, kind="ExternalOutput")

    # DRAM views with the c-half dim split out so one DMA covers both halves
    cp2 = cp[:].rearrange("(u p) n -> p u n", u=2)
    x2 = x[:].rearrange("(u p) n -> p u n", u=2)
    y2 = y[:].rearrange("(u p) n -> p u n", u=2)

    with tile.TileContext(nc) as tc:
        with (
            tc.tile_pool(name="const", bufs=1) as cst,
            tc.tile_pool(name="big", bufs=1) as big,
            tc.tile_pool(name="eqp", bufs=4) as eqp,
            tc.tile_pool(name="rdp", bufs=3) as rdp,
            tc.tile_pool(name="qtp", bufs=3) as qtp,
            tc.tile_pool(name="o2p", bufs=4) as o2p,
        ):
            # --- warmup fodder: zeroed tile, no input dependencies ---
            zt = cst.tile([128, 512], SDT, name="zt")
            nc.vector.memset(zt[:].bitcast(F32), 0.0)

            # --- head-indicator block-diagonal matrix (identical for both
            # c-halves: heads are 64 channels, aligned within each 128-half)
            amat = cst.tile([128, 128], SDT, name="amat")
            nc.vector.memset(amat[:].bitcast(F32), 0.0)
            nc.vector.memset(amat[0:64, 0:64].bitcast(F32), 1.0)
            nc.vector.memset(amat[64:128, 64:128].bitcast(F32), 1.0)

            # manually-rotated V^T ring: ones columns pre-set once.  memset
            # writes a 4-byte pattern: f32 1.0, or a packed pair of bf16 1.0s.
            NVBUF = 4
            ones_pat = (
                float(np.frombuffer(np.uint32(0x3F803F80).tobytes(), np.float32)[0])
                if KDT == BF16
                else 1.0
            )
            v2r = [cst.tile([128, 2 * VW], KDT, name=f"v2_{i}") for i in range(NVBUF)]
            for i in range(NVBUF):
                for h in range(2):
                    o = v2r[i][:, h * VW + C : h * VW + C + 2].bitcast(F32)
                    nc.vector.memset(o, ones_pat)

            # --- packed weights; the KV-phase slice (wkvT) rides first ---
            wpk = cst.tile([128, 2 * WPB], IDT, name="wpk")
            wpk3 = wpk[:].rearrange("p (u w) -> p u w", u=2)
            wpack3 = wpack[:].rearrange("p (u w) -> p u w", u=2)
            wof = cst.tile([128, 2 * C], SDT, name="wof")
            wkvT_sb = [wpk[:, u * WPB : u * WPB + 2 * C] for u in range(2)]
            wqT_sb = [wpk[:, u * WPB + 2 * C : u * WPB + 3 * C] for u in range(2)]
            woT_sb = [wof[:, u * C : (u + 1) * C] for u in range(2)]

            cf_sb = big.tile([128, 2, N], IDT, name="cf_sb")
            xf_sb = big.tile([128, 2, N], IDT, name="xf_sb")

            # --- input triggers: early DMA runs at only ~130 GB/s aggregate,
            # so the first-iteration gating set (wkvT both halves + first cf
            # slices) is split ACROSS the two HWDGE queues (sync + scalar) so
            # the halves land in parallel, and the cf gates are 128-col
            # per-c-half slices (32 KB).  Only SP and ACT have HWDGE queues.
            nc.sync.dma_start(out=wpk3[:, 0, 0 : 2 * C], in_=wpack3[:, 0, 0 : 2 * C])
            nc.scalar.dma_start(out=wpk3[:, 1, 0 : 2 * C], in_=wpack3[:, 1, 0 : 2 * C])
            nc.sync.dma_start(out=cf_sb[:, 0:1, 0:128], in_=cp2[:, 0:1, 0:128])
            nc.scalar.dma_start(out=cf_sb[:, 1:2, 0:128], in_=cp2[:, 1:2, 0:128])
            nc.sync.dma_start(out=cf_sb[:, 0:1, 128:256], in_=cp2[:, 0:1, 128:256])
            nc.scalar.dma_start(out=cf_sb[:, 1:2, 128:256], in_=cp2[:, 1:2, 128:256])
            nc.scalar.dma_start(out=cf_sb[:, :, 256:512], in_=cp2[:, :, 256:512])
            # bulk: cf then x, graduated, all on sync (scalar goes back to ACT
            # work; wqT/wof triggers are issued mid-KV-loop below)
            nc.sync.dma_start(out=cf_sb[:, :, 512:1024], in_=cp2[:, :, 512:1024])
            nc.sync.dma_start(out=cf_sb[:, :, 1024:2048], in_=cp2[:, :, 1024:2048])
            nc.sync.dma_start(out=cf_sb[:, :, 2048:4096], in_=cp2[:, :, 2048:4096])
            nc.sync.dma_start(out=xf_sb[:, :, 0:2048], in_=x2[:, :, 0:2048])
            nc.sync.dma_start(out=xf_sb[:, :, 2048:4096], in_=x2[:, :, 2048:4096])
            if use_bq:
                bq_sb = [cst.tile([128, 1], F32, name=f"bq{u}") for u in range(2)]
                for u in range(2):
                    nc.scalar.dma_start(out=bq_sb[u][:], in_=bq_s[ts(u, 128), :])
            if use_bo:
                bo_sb = [cst.tile([128, 1], F32, name=f"bo{u}") for u in range(2)]
                for u in range(2):
                    nc.scalar.dma_start(out=bo_sb[u][:], in_=bo_c[ts(u, 128), :])
            if use_bv:
                bv_sb = cst.tile([1, C], SDT, name="bv_sb")
                nc.scalar.dma_start(out=bv_sb[:], in_=bv_r[:])
                wosum_sb = cst.tile([1, C], SDT, name="wosum_sb")
                nc.scalar.dma_start(out=wosum_sb[:], in_=wosum[:])

            # persistent W^T tiles (filled in the epilogue)
            WT_sb = [cst.tile([128, C], SDT, name=f"WT{u}") for u in range(2)]

            eqs, psDs, qts = {}, {}, {}

            def q_mms_into(j, psQ):
                for t in range(2):
                    for u in range(2):
                        nc.tensor.matmul(
                            psQ[:, t * 512 : (t + 1) * 512],
                            wqT_sb[u][:, ts(t, 128)],
                            xf_sb[:, u, ts(j, 512)],
                            start=(u == 0),
                            stop=(u == 1),
                        )

            def eq_act(j, psQ):
                eq = eqp.tile([128, 1024], SDT, name="eq", tag="eq")
                if use_bq:
                    for t in range(2):
                        nc.scalar.activation(
                            out=eq[:, t * 512 : (t + 1) * 512],
                            in_=psQ[:, t * 512 : (t + 1) * 512],
                            func=AF.Exp,
                            scale=SCALE,
                            bias=bq_sb[t][:],
                        )
                else:
                    nc.scalar.activation(
                        out=eq[:], in_=psQ[:], func=AF.Exp, scale=SCALE
                    )
                eqs[j] = eq

            def dfull_mms(j, pool=None, tag="psD"):
                psD = (pool or psdp).tile([128, 1024], F32, name="psD", tag=tag)
                eq = eqs[j]
                for t in range(2):
                    nc.tensor.matmul(
                        psD[:, t * 512 : (t + 1) * 512],
                        amat[:],
                        eq[:, t * 512 : (t + 1) * 512],
                        start=True,
                        stop=True,
                    )
                psDs[j] = psD

            def recip_qmul(j):
                # qt = eq * (1/D).  The TT-ALU has no divide (s3s3d3 ISA
                # check), so it's the single-instruction approx reciprocal
                # plus a multiply.  GpSimd offload measured 2x slower AND it
                # shares the DVE SBUF port — everything rides DVE.
                psD = psDs.pop(j)
                rD = rdp.tile([128, 1024], F32, name="rD", tag="rD")
                nc.vector.reciprocal_approx_fast(out=rD[:], in_=psD[:])
                qt = qtp.tile([128, 1024], SDT, name="qt", tag="qt")
                eq = eqs.pop(j)
                nc.vector.tensor_mul(qt[:], eq[:], rD[:])
                qts[j] = qt

            # ---- warmup: keep the PE (and the HAM busy-window) running from
            # right after the preamble until the first input data lands ----
            pwarm = tc.alloc_tile_pool(name="pswarm", bufs=1, space="PSUM")
            psw = pwarm.tile([128, 512], F32, name="psw")
            for k in range(NWARM):
                nc.tensor.matmul(
                    psw[:], zt[:, 0:128], zt[:],
                    start=True, stop=True, skip_group_check=True,
                )
            pwarm.release()

            # ============ KV phase: context = exp(K) @ [V^T | 1] ============
            # Software-pipelined by one iteration: the PE runs iteration i's
            # KV matmuls and iteration i-1's context matmuls back to back.
            # u is the outer loop so iteration 0's first matmuls gate only on
            # the per-c-half 128-col cf slices.
            pctx = tc.alloc_tile_pool(name="psum_ctx", bufs=1, space="PSUM")
            psCtx = [pctx.tile([128, VW], F32, name=f"psCtx{u}") for u in range(2)]
            pkv = tc.alloc_tile_pool(name="psum_kv", bufs=3, space="PSUM")
            kvsb = tc.alloc_tile_pool(name="kvsb", bufs=3)
            eks = {}

            def kv_mms(i):
                psKV = pkv.tile([128, 1024], F32, name="psKV")
                for u in range(2):
                    for h in range(2):
                        nt = 2 * i + h
                        nc.tensor.matmul(
                            psKV[:, h * 512 : (h + 1) * 512],
                            cf_sb[:, u, ts(nt, 128)],
                            wkvT_sb[u],
                            start=(u == 0),
                            stop=(u == 1),
                            skip_group_check=True,
                        )
                return psKV

            def ctx_mms(i):
                ek = eks.pop(i)
                v2 = v2r[i % NVBUF]
                for h in range(2):
                    for u in range(2):
                        nc.tensor.matmul(
                            psCtx[u][:],
                            ek[:, h, ts(u, 128)],
                            v2[:, h * VW : (h + 1) * VW],
                            start=(i == 0 and h == 0),
                            stop=(i == NSUPER - 1 and h == 1),
                            skip_group_check=True,
                        )

            def kv_post(i, psKV):
                ek = kvsb.tile([128, 2, C], KDT, name="ek")
                nc.scalar.activation(
                    out=ek[:],
                    in_=psKV[:].rearrange("p (h c) -> p h c", h=2)[:, :, 0:C],
                    func=AF.Exp,
                )
                eks[i] = ek
                v2 = v2r[i % NVBUF]
                nc.vector.tensor_copy(
                    v2[:].rearrange("p (h w) -> p h w", h=2)[:, :, 0:C],
                    psKV[:].rearrange("p (h c) -> p h c", h=2)[:, :, C : 2 * C],
                )

            # the first two Q chunks ride inside the KV tail (their PSUM
            # supertiles borrow the KV pool's slots) so their eq tiles are
            # ready the moment the KV phase ends
            for i in range(NSUPER):
                psKV = kv_mms(i)
                if i > 0:
                    ctx_mms(i - 1)
                kv_post(i, psKV)
                if i == 2:
                    # wqT (needed from iter 14) + woT (epilogue): issue these
                    # scalar-queue triggers after the first ek ACTs so they
                    # don't delay the KV pipeline's scalar work
                    nc.scalar.dma_start(
                        out=wpk3[:, :, 2 * C : WPB], in_=wpack3[:, :, 2 * C : WPB]
                    )
                    nc.scalar.dma_start(out=wof[:], in_=wof_d[:])
                if i >= NSUPER - 2:
                    psQ = pkv.tile([128, 1024], F32, name="psKV", tag="psKV")
                    q_mms_into(i - (NSUPER - 2), psQ)
                    eq_act(i - (NSUPER - 2), psQ)
            ctx_mms(NSUPER - 1)
            kvsb.release()
            pkv.release()

            # ===== epilogue part 1: normalize context rows.  Only the tiny
            # [128,1] reciprocals ride DVE; the scaling copy goes to ACT
            # (per-partition scale vector) so the DVE can start the Q-phase
            # recip/qmul chain — the critical serial path — immediately.
            rcol = [cst.tile([128, 1], F32, name=f"rcol{u}") for u in range(2)]
            ctx_sb = [cst.tile([128, C], SDT, name=f"ctx{u}") for u in range(2)]
            for u in range(2):
                nc.vector.reciprocal(rcol[u][:], psCtx[u][:, C : C + 1])
            for u in range(2):
                nc.scalar.activation(
                    out=ctx_sb[u][:], in_=psCtx[u][:, 0:C],
                    func=AF.Copy, scale=rcol[u][:],
                )
            pctx.release()

            # pool stack is LIFO: psq/psd/po/pw pushed after pctx popped.
            # (Program order doesn't serialize engines — the PE still runs
            # dfull(0) right after ctx(15); only data deps matter.)
            psqp = tc.alloc_tile_pool(name="psq", bufs=1, space="PSUM")
            psdp = tc.alloc_tile_pool(name="psd", bufs=1, space="PSUM")
            po = tc.alloc_tile_pool(name="pso", bufs=1, space="PSUM")
            pw = tc.alloc_tile_pool(name="psum_w", bufs=1, space="PSUM")

            # chunks 0-1's softmax chains run across the seam, shortening the
            # post-KV DVE serial chain (the Q phase's critical path).  The
            # W-fold matmuls sit between them so the PE never waits on the
            # psd WAR (dfull(1) needs recip(0) to have drained psD).
            dfull_mms(0)
            recip_qmul(0)

            # ===== epilogue part 2: fold wo, W^T = ctx.T @ woT =====
            psW = [pw.tile([128, C], F32, name=f"psW{v}") for v in range(2)]
            for v in range(2):
                for u in range(2):
                    nc.tensor.matmul(
                        psW[v][:],
                        ctx_sb[u][:, ts(v, 128)],
                        woT_sb[u],
                        start=(u == 0),
                        stop=(u == 1) and not use_bv,
                        skip_group_check=True,
                    )
                if use_bv:
                    # context gains +bv[d'] per row (sum_n k = 1), so
                    # W^T += bv (X) rowsum(wo): a K=1 rank-1 matmul.
                    nc.tensor.matmul(
                        psW[v][:],
                        bv_sb[:, ts(v, 128)],
                        wosum_sb[:],
                        start=False,
                        stop=True,
                        skip_group_check=True,
                    )
                # WT copy on ACT — the DVE is saturated at the seam with the
                # hoisted recip/qmul chains
                nc.scalar.activation(out=WT_sb[v][:], in_=psW[v][:], func=AF.Copy)
            dfull_mms(1)
            recip_qmul(1)

            def out_mms(c, psO):
                qt = qts.pop(c)
                for t in range(2):
                    for u in range(2):
                        nc.tensor.matmul(
                            psO[:, t * 512 : (t + 1) * 512],
                            WT_sb[u][:, ts(t, 128)],
                            qt[:, u * 512 : (u + 1) * 512],
                            start=(u == 0),
                            stop=(u == 1),
                        )

            def out_copy_store(c, psO):
                o2 = o2p.tile([128, 2, 512], ODT, name="o2", tag="o2")
                o2f = o2[:].rearrange("p t n -> p (t n)")
                if use_bo:
                    for t in range(2):
                        nc.vector.tensor_scalar_add(
                            out=o2[:, t, :],
                            in0=psO[:, t * 512 : (t + 1) * 512],
                            scalar1=bo_sb[t][:],
                        )
                    nc.sync.dma_start(out=y2[:, :, ts(c, 512)], in_=o2[:])
                    return
                # o2's flat free layout matches psO's supertile order, so ONE
                # full-width ACT copy does the whole cast (ACT has the slack;
                # DVE is the critical engine in the Q loop).  The last chunk
                # instead splits copy ACT/DVE (DVE is done by then) and its
                # halves ride two DMA queues, so the final store drains fast.
                if c == NCHUNKS - 1:
                    nc.scalar.activation(
                        out=o2[:, 0, :], in_=psO[:, 0:512], func=AF.Copy
                    )
                    nc.sync.dma_start(out=y2[:, 0:1, ts(c, 512)], in_=o2[:, 0:1, :])
                    nc.vector.tensor_copy(o2[:, 1, :], psO[:, 512:1024])
                    nc.scalar.dma_start(out=y2[:, 1:2, ts(c, 512)], in_=o2[:, 1:2, :])
                else:
                    nc.scalar.activation(out=o2f, in_=psO[:], func=AF.Copy)
                    nc.sync.dma_start(out=y2[:, :, ts(c, 512)], in_=o2[:])

            # ===== Q main loop: stages q(j), eq(j), Dfull(j-2), recip/qmul
            # (j-2), out(j-3), copy+store(j-3).  Chunks 0-1's q/eq ran in the
            # KV tail; chunk 0's Dfull/recip/qmul ran in the epilogue. =====
            po2 = None
            psOs = {}
            for j in range(2, NCHUNKS + 3):
                if 2 <= j - 2 <= NCHUNKS - 1:
                    dfull_mms(j - 2)
                if j - 3 >= 0:
                    c = j - 3
                    pool = po if c % 2 == 0 else po2
                    psO = pool.tile([128, 1024], F32, name="psO", tag="psO")
                    out_mms(c, psO)
                    psOs[c] = psO
                if j <= NCHUNKS - 1:
                    psQ = psqp.tile([128, 1024], F32, name="psQ", tag="psQ")
                    q_mms_into(j, psQ)
                    eq_act(j, psQ)
                if 2 <= j - 2 <= NCHUNKS - 1:
                    recip_qmul(j - 2)
                if j - 3 >= 0:
                    out_copy_store(j - 3, psOs.pop(j - 3))
                if j == 2:
                    # pw's banks free after the WT copies -> second out pool
                    pw.release()
                    po2 = tc.alloc_tile_pool(name="pso2", bufs=1, space="PSUM")
            po2.release()
            po.release()
            psdp.release()
            psqp.release()

    nc.compile()
    return nc


def _get_nc(use_bq, use_bo, use_bv, mm_dtype):
    key = (use_bq, use_bo, use_bv, str(mm_dtype))
    if key not in _CACHE:
        with _single_act_table():
            _CACHE[key] = _build(use_bq, use_bo, use_bv, mm_dtype)
    return _CACHE[key]


def _to_mdt(a, mm_dtype):
    if mm_dtype == BF16:
        import ml_dtypes

        return np.ascontiguousarray(a.astype(ml_dtypes.bfloat16))
    return np.ascontiguousarray(a)


def kernel(x, cproj, wq, bq, wkv, bkv, wo, bo, _mm_dtype=BF16, _results_hook=None):
    x = np.ascontiguousarray(np.asarray(x, dtype=np.float32).reshape(B, C, N))
    cf = np.ascontiguousarray(np.asarray(cproj, dtype=np.float32).reshape(B, C, N))
    wq = np.asarray(wq, dtype=np.float32)
    wkv = np.asarray(wkv, dtype=np.float32)
    wo = np.asarray(wo, dtype=np.float32)
    bq = np.asarray(bq, dtype=np.float32)
    bkv = np.asarray(bkv, dtype=np.float32)
    bo = np.asarray(bo, dtype=np.float32)

    use_bq = bool(np.any(bq != 0))
    use_bo = bool(np.any(bo != 0))
    bv = bkv[C:]
    use_bv = bool(np.any(bv != 0))

    wqT = np.ascontiguousarray(wq.T)
    wkvT = np.ascontiguousarray(wkv.T)
    woT = np.ascontiguousarray(wo.T)

    # packed weights: bf16 [wkvT | wqT] per c-half, f32 [woT] per c-half
    wpack = np.zeros((128, 2 * WPB), np.float32)
    wof = np.zeros((128, 2 * C), np.float32)
    for u in range(2):
        r = slice(u * 128, (u + 1) * 128)
        wpack[:, u * WPB : u * WPB + 2 * C] = wkvT[r]
        wpack[:, u * WPB + 2 * C : u * WPB + 3 * C] = wqT[r]
        wof[:, u * C : (u + 1) * C] = woT[r]

    nc = _get_nc(use_bq, use_bo, use_bv, _mm_dtype)

    base = {
        "wpack": _to_mdt(wpack, _mm_dtype),
        "wof": wof,
    }
    if use_bq:
        base["bq_s"] = (SCALE * bq).reshape(C, 1)
    if use_bo:
        base["bo_c"] = bo.reshape(C, 1)
    if use_bv:
        base["bv_r"] = bv.reshape(1, C)
        base["wosum"] = wo.sum(axis=1).reshape(1, C)

    in_maps = [
        dict(base, x=_to_mdt(x[b], _mm_dtype), cp=_to_mdt(cf[b], _mm_dtype))
        for b in range(B)
    ]
    res = run_bass_kernel_spmd(nc, in_maps, list(range(NCORES)))
    if _results_hook is not None:
        _results_hook(res)
    out = np.stack(
        [np.asarray(res.results[b]["y"], dtype=np.float32) for b in range(B)],
        axis=0,
    )
    return out.reshape(B, C, H, W)


# revision 32
# speedup vs baseline: 1.1666x; 1.0128x over previous
"""Trainium2 Bass kernel for CrossEfficientAttention (B=8, C=256, H=W=64, 4 heads).

Sharding: data-parallel over batch B — one sample per NeuronCore, no collectives.

Per-core math (sample x_s, c_s of shape [C, N], N = H*W = 4096):
    Q  = wq @ x_s                      (+ bq, folded into the exp's ACT bias)
    KV = wkv @ c_s                     (bkv[:C] cancels exactly in softmax over N;
                                        bkv[C:] handled as a rank-1 update of W)
    k  = softmax_N(K); q = softmax_head(Q * C**-0.25)
    context = k @ V^T ; out = wo @ (context @ q) + bo

Restructured for the PE array (out = lhsT.T @ rhs, contraction over partitions):
  * KV^T computed directly in [N, C] layout by using c_s tiles as lhsT.
  * k-softmax normalizer: ones-columns appended to V^T give row sums of exp(K)
    in column 256 of the context PSUM accumulator; context rows are then scaled
    by the reciprocal column (per-partition tensor_scalar) — no transposes.
  * wo folded in early: W^T = matmul(lhsT=context, rhs=wo^T) directly in [d, o]
    layout. The per-chunk output is then just out2 = W^T.T @ q.
  * q-softmax denominators via a block-diagonal ones matrix ("Amat",
    blockdiag(J64, J64), identical for both channel halves): Dfull = Amat.T @ eq
    puts D[head(c), n] at every partition c directly — no 4-row D tile, no
    Ln/Exp round-trip, no selector-broadcast matmul.  1/D via the single-
    instruction DVE reciprocal_approx_fast (~51 ULP), and q = eq * rD with the
    multiply split DVE/GpSimd so no single engine becomes the bottleneck.
  * Q phase paced evenly: at iteration j the PE runs Dfull(j-2), out(j-3),
    q(j) back to back; ACT does exp(j) + half of copy(j-3); DVE does
    recip(j-2), half of qmul(j-2), half of copy(j-3); GpSimd does the other
    qmul half.  No long serialized drain — the last chunk's store follows its
    matmuls by ~1.5us, and its two halves ride two DMA queues.

Hard-won scheduling facts (measured on hardware):
  * The PE HAM clock gate passes 4/8 pulses (1.2 GHz) until it has seen a full
    ~3.4us busy window, then 8/8 (2.4 GHz).  The framework preamble takes
    ~6.6us and the first input chunk lands ~2.5us after its trigger, so with
    no warmup the first ~6us of real matmuls run at half clock.  Warmup
    matmuls on a zeroed SBUF tile, sized to end just as the first data lands,
    start the busy window early so real work runs warm almost immediately.
  * DMA trigger instructions cost ~0.65us of issuing-engine queue time each,
    and the first bytes of a queue move ~1.6us after the trigger.  Packet
    count per transfer is rows/16-queues; a 128-col one-c-half slice of cf is
    8 packets/queue and lands ~2us after its trigger.
  * Inputs (x, cp, wkv, wq) and the y output ride in bf16 — halves both
    the input stream and the store stream for ~4.1e-3 rel err; all on-chip
    intermediates stay f32r (same PE speed). fp8 measured 4.6e-2 rel err
    (per-element quantization noise does not average out in the cancelling
    context/out sums), well over the 2e-2 gate.
  * An SBUF->SBUF broadcast DMA (stride-0 source) crashed the device
    (NRT_EXEC_UNIT_UNRECOVERABLE) — broadcasts go through the PE.
"""

import contextlib

import numpy as np

import concourse.bass as bass
import concourse.tile as tile
from concourse import bacc, mybir
from concourse.bass import ts
from concourse.bass_utils import run_bass_kernel_spmd

B, C, H, W = 8, 256, 64, 64
N = H * W
NHEADS = 4
DHEAD = C // NHEADS
NCORES = 8
NSUPER = N // 256          # 16 double-n-tile iterations for the KV phase
NCHUNKS = N // 512         # 8 column chunks for the Q/output phase
SCALE = float(1.0 / np.sqrt(np.sqrt(np.float32(C))))
VW = C + 2                 # V^T tile row width (256 data + 2 ones cols; fp32r needs even free)
WPB = 2 * C + C            # bf16 packed row width per c-half: wkvT|wqT
NWARM = 5                  # warmup matmuls (N=512 cold ~= 0.61us each).  The
                           # first gating DMAs land anywhere in 11-15us run to
                           # run, so dummy-LDWEIGHTS filler (107ns each, no
                           # PSUM writes, safe between matmul groups because
                           # matmuls self-load their weights) bridges the
                           # variance and keeps the HAM busy-window filling.

F32 = mybir.dt.float32
F32R = mybir.dt.float32r
BF16 = mybir.dt.bfloat16
AF = mybir.ActivationFunctionType

_CACHE = {}


def _single_act_table():
    """Scope-patch the activation-table list so the table-load pass resolves
    Exp (and Ln, unused now) to natural_log_exp_and_others (set ids stay
    positional, so only the function lists may change, not the order)."""
    import concourse.bacc as cbacc
    from concourse.hw_specs import get_activation_tables

    @contextlib.contextmanager
    def scope():
        orig = cbacc.get_activation_tables

        def patched(arch):
            tabs = get_activation_tables(arch)
            return {
                k: (v if k == "natural_log_exp_and_others" else set())
                for k, v in tabs.items()
            }

        cbacc.get_activation_tables = patched
        try:
            yield
        finally:
            cbacc.get_activation_tables = orig

    return scope()


def _build(use_bq, use_bo, use_bv, mm_dtype):
    nc = bacc.Bacc("TRN2", target_bir_lowering=False, debug=False)
    # IDT: dtype of the DMA-heavy inputs (x, cp, wkv, wq) — bf16 halves the
    # input stream with one rounding ahead of the softmax averaging.
    # SDT: on-chip intermediates stay f32r.
    IDT = mm_dtype
    SDT = F32R
    # KV-context operands (exp(K), V^T) in bf16: the 64 ctx matmuls drop from
    # ~125ns to ~110ns each and the KV phase ends ~1us earlier (everything
    # downstream is serialized behind it).  ~+0.5% rel err via the cancelling
    # context sum — budget is 2e-2, measured ~4e-3 before this.
    KDT = BF16 if mm_dtype == BF16 else F32R

    x = nc.dram_tensor("x", [C, N], IDT, kind="ExternalInput")
    cp = nc.dram_tensor("cp", [C, N], IDT, kind="ExternalInput")
    wpack = nc.dram_tensor("wpack", [128, 2 * WPB], IDT, kind="ExternalInput")
    wof_d = nc.dram_tensor("wof", [128, 2 * C], F32R, kind="ExternalInput")
    if use_bq:
        bq_s = nc.dram_tensor("bq_s", [C, 1], F32, kind="ExternalInput")
    if use_bo:
        bo_c = nc.dram_tensor("bo_c", [C, 1], F32, kind="ExternalInput")
    if use_bv:
        bv_r = nc.dram_tensor("bv_r", [1, C], F32R, kind="ExternalInput")
        wosum = nc.dram_tensor("wosum", [1, C], F32R, kind="ExternalInput")
    ODT = BF16 if mm_dtype == BF16 else F32
    y = nc.dram_tensor("y", [C, N], ODT, kind="ExternalOutput")

    # DRAM views with the c-half dim split out so one DMA covers both halves
    cp2 = cp[:].rearrange("(u p) n -> p u n", u=2)
    x2 = x[:].rearrange("(u p) n -> p u n", u=2)
    y2 = y[:].rearrange("(u p) n -> p u n", u=2)

    with tile.TileContext(nc) as tc:
        with (
            tc.tile_pool(name="const", bufs=1) as cst,
            tc.tile_pool(name="big", bufs=1) as big,
            tc.tile_pool(name="eqp", bufs=4) as eqp,
            tc.tile_pool(name="rdp", bufs=3) as rdp,
            tc.tile_pool(name="qtp", bufs=3) as qtp,
            tc.tile_pool(name="o2p", bufs=4) as o2p,
        ):
            # --- warmup fodder: zeroed tile, no input dependencies
            # (bf16: ldweights() rejects f32/f32r) ---
            zt = cst.tile([128, 512], BF16, name="zt")
            nc.vector.memset(zt[:].bitcast(F32), 0.0)

            def warm_ldw(n):
                # dummy weight loads: PE-busy filler with no PSUM side
                # effects; each delays real work by at most ~107ns
                for _ in range(n):
                    nc.tensor.ldweights(zt[:, 0:128])

            # --- head-indicator block-diagonal matrix (identical for both
            # c-halves: heads are 64 channels, aligned within each 128-half)
            amat = cst.tile([128, 128], SDT, name="amat")
            nc.vector.memset(amat[:].bitcast(F32), 0.0)
            nc.vector.memset(amat[0:64, 0:64].bitcast(F32), 1.0)
            nc.vector.memset(amat[64:128, 64:128].bitcast(F32), 1.0)

            # manually-rotated V^T ring: ones columns pre-set once.  memset
            # writes a 4-byte pattern: f32 1.0, or a packed pair of bf16 1.0s.
            NVBUF = 4
            ones_pat = (
                float(np.frombuffer(np.uint32(0x3F803F80).tobytes(), np.float32)[0])
                if KDT == BF16
                else 1.0
            )
            v2r = [cst.tile([128, 2 * VW], KDT, name=f"v2_{i}") for i in range(NVBUF)]
            for i in range(NVBUF):
                for h in range(2):
                    o = v2r[i][:, h * VW + C : h * VW + C + 2].bitcast(F32)
                    nc.vector.memset(o, ones_pat)

            # --- packed weights; the KV-phase slice (wkvT) rides first ---
            wpk = cst.tile([128, 2 * WPB], IDT, name="wpk")
            wpk3 = wpk[:].rearrange("p (u w) -> p u w", u=2)
            wpack3 = wpack[:].rearrange("p (u w) -> p u w", u=2)
            wof = cst.tile([128, 2 * C], SDT, name="wof")
            wkvT_sb = [wpk[:, u * WPB : u * WPB + 2 * C] for u in range(2)]
            wqT_sb = [wpk[:, u * WPB + 2 * C : u * WPB + 3 * C] for u in range(2)]
            woT_sb = [wof[:, u * C : (u + 1) * C] for u in range(2)]

            cf_sb = big.tile([128, 2, N], IDT, name="cf_sb")
            xf_sb = big.tile([128, 2, N], IDT, name="xf_sb")

            # --- input triggers: early DMA runs at only ~130 GB/s aggregate,
            # so the first-iteration gating set (wkvT both halves + first cf
            # slices) is split ACROSS the two HWDGE queues (sync + scalar) so
            # the halves land in parallel, and the cf gates are 128-col
            # per-c-half slices (32 KB).  Only SP and ACT have HWDGE queues.
            nc.sync.dma_start(out=wpk3[:, 0, 0 : 2 * C], in_=wpack3[:, 0, 0 : 2 * C])
            nc.scalar.dma_start(out=wpk3[:, 1, 0 : 2 * C], in_=wpack3[:, 1, 0 : 2 * C])
            nc.sync.dma_start(out=cf_sb[:, 0:1, 0:128], in_=cp2[:, 0:1, 0:128])
            nc.scalar.dma_start(out=cf_sb[:, 1:2, 0:128], in_=cp2[:, 1:2, 0:128])
            nc.sync.dma_start(out=cf_sb[:, 0:1, 128:256], in_=cp2[:, 0:1, 128:256])
            nc.scalar.dma_start(out=cf_sb[:, 1:2, 128:256], in_=cp2[:, 1:2, 128:256])
            nc.scalar.dma_start(out=cf_sb[:, :, 256:512], in_=cp2[:, :, 256:512])
            # bulk: cf then x, graduated, all on sync (scalar goes back to ACT
            # work; wqT/wof triggers are issued mid-KV-loop below)
            nc.sync.dma_start(out=cf_sb[:, :, 512:1024], in_=cp2[:, :, 512:1024])
            nc.sync.dma_start(out=cf_sb[:, :, 1024:2048], in_=cp2[:, :, 1024:2048])
            nc.sync.dma_start(out=cf_sb[:, :, 2048:4096], in_=cp2[:, :, 2048:4096])
            nc.sync.dma_start(out=xf_sb[:, :, 0:2048], in_=x2[:, :, 0:2048])
            nc.sync.dma_start(out=xf_sb[:, :, 2048:4096], in_=x2[:, :, 2048:4096])
            if use_bq:
                bq_sb = [cst.tile([128, 1], F32, name=f"bq{u}") for u in range(2)]
                for u in range(2):
                    nc.scalar.dma_start(out=bq_sb[u][:], in_=bq_s[ts(u, 128), :])
            if use_bo:
                bo_sb = [cst.tile([128, 1], F32, name=f"bo{u}") for u in range(2)]
                for u in range(2):
                    nc.scalar.dma_start(out=bo_sb[u][:], in_=bo_c[ts(u, 128), :])
            if use_bv:
                bv_sb = cst.tile([1, C], SDT, name="bv_sb")
                nc.scalar.dma_start(out=bv_sb[:], in_=bv_r[:])
                wosum_sb = cst.tile([1, C], SDT, name="wosum_sb")
                nc.scalar.dma_start(out=wosum_sb[:], in_=wosum[:])

            # persistent W^T tiles (filled in the epilogue)
            WT_sb = [cst.tile([128, C], SDT, name=f"WT{u}") for u in range(2)]

            eqs, psDs, qts = {}, {}, {}

            def q_mms_into(j, psQ):
                for t in range(2):
                    for u in range(2):
                        nc.tensor.matmul(
                            psQ[:, t * 512 : (t + 1) * 512],
                            wqT_sb[u][:, ts(t, 128)],
                            xf_sb[:, u, ts(j, 512)],
                            start=(u == 0),
                            stop=(u == 1),
                        )

            def eq_act(j, psQ):
                eq = eqp.tile([128, 1024], SDT, name="eq", tag="eq")
                if use_bq:
                    for t in range(2):
                        nc.scalar.activation(
                            out=eq[:, t * 512 : (t + 1) * 512],
                            in_=psQ[:, t * 512 : (t + 1) * 512],
                            func=AF.Exp,
                            scale=SCALE,
                            bias=bq_sb[t][:],
                        )
                else:
                    nc.scalar.activation(
                        out=eq[:], in_=psQ[:], func=AF.Exp, scale=SCALE
                    )
                eqs[j] = eq

            def dfull_mms(j, pool=None, tag="psD"):
                psD = (pool or psdp).tile([128, 1024], F32, name="psD", tag=tag)
                eq = eqs[j]
                for t in range(2):
                    nc.tensor.matmul(
                        psD[:, t * 512 : (t + 1) * 512],
                        amat[:],
                        eq[:, t * 512 : (t + 1) * 512],
                        start=True,
                        stop=True,
                    )
                psDs[j] = psD

            def recip_qmul(j):
                # qt = eq * (1/D).  The TT-ALU has no divide (s3s3d3 ISA
                # check), so it's the single-instruction approx reciprocal
                # plus a multiply.  GpSimd offload measured 2x slower AND it
                # shares the DVE SBUF port — everything rides DVE.
                psD = psDs.pop(j)
                rD = rdp.tile([128, 1024], F32, name="rD", tag="rD")
                nc.vector.reciprocal_approx_fast(out=rD[:], in_=psD[:])
                qt = qtp.tile([128, 1024], SDT, name="qt", tag="qt")
                eq = eqs.pop(j)
                nc.vector.tensor_mul(qt[:], eq[:], rD[:])
                qts[j] = qt

            # ---- warmup: keep the PE (and the HAM busy-window) running from
            # right after the preamble until the first input data lands ----
            pwarm = tc.alloc_tile_pool(name="pswarm", bufs=1, space="PSUM")
            psw = pwarm.tile([128, 512], F32, name="psw")
            for k in range(NWARM):
                nc.tensor.matmul(
                    psw[:], zt[:, 0:128], zt[:],
                    start=True, stop=True, skip_group_check=True,
                )
            warm_ldw(10)
            pwarm.release()

            # ============ KV phase: context = exp(K) @ [V^T | 1] ============
            # Software-pipelined by one iteration: the PE runs iteration i's
            # KV matmuls and iteration i-1's context matmuls back to back.
            # u is the outer loop so iteration 0's first matmuls gate only on
            # the per-c-half 128-col cf slices.
            pctx = tc.alloc_tile_pool(name="psum_ctx", bufs=1, space="PSUM")
            psCtx = [pctx.tile([128, VW], F32, name=f"psCtx{u}") for u in range(2)]
            pkv = tc.alloc_tile_pool(name="psum_kv", bufs=3, space="PSUM")
            kvsb = tc.alloc_tile_pool(name="kvsb", bufs=3)
            eks = {}

            def kv_mms(i):
                psKV = pkv.tile([128, 1024], F32, name="psKV")
                for u in range(2):
                    for h in range(2):
                        nt = 2 * i + h
                        nc.tensor.matmul(
                            psKV[:, h * 512 : (h + 1) * 512],
                            cf_sb[:, u, ts(nt, 128)],
                            wkvT_sb[u],
                            start=(u == 0),
                            stop=(u == 1),
                            skip_group_check=True,
                        )
                return psKV

            def ctx_mms(i):
                ek = eks.pop(i)
                v2 = v2r[i % NVBUF]
                for h in range(2):
                    for u in range(2):
                        nc.tensor.matmul(
                            psCtx[u][:],
                            ek[:, h, ts(u, 128)],
                            v2[:, h * VW : (h + 1) * VW],
                            start=(i == 0 and h == 0),
                            stop=(i == NSUPER - 1 and h == 1),
                            skip_group_check=True,
                        )

            def kv_post(i, psKV):
                ek = kvsb.tile([128, 2, C], KDT, name="ek")
                nc.scalar.activation(
                    out=ek[:],
                    in_=psKV[:].rearrange("p (h c) -> p h c", h=2)[:, :, 0:C],
                    func=AF.Exp,
                )
                eks[i] = ek
                v2 = v2r[i % NVBUF]
                nc.vector.tensor_copy(
                    v2[:].rearrange("p (h w) -> p h w", h=2)[:, :, 0:C],
                    psKV[:].rearrange("p (h c) -> p h c", h=2)[:, :, C : 2 * C],
                )

            # the first two Q chunks ride inside the KV tail (their PSUM
            # supertiles borrow the KV pool's slots) so their eq tiles are
            # ready the moment the KV phase ends
            for i in range(NSUPER):
                psKV = kv_mms(i)
                if i > 0:
                    ctx_mms(i - 1)
                kv_post(i, psKV)
                if i <= 3:
                    # early iterations are DMA-dribble-paced; keep the HAM
                    # busy-window filling across the data waits
                    warm_ldw(6)
                if i == 2:
                    # wqT (needed from iter 14) + woT (epilogue): issue these
                    # scalar-queue triggers after the first ek ACTs so they
                    # don't delay the KV pipeline's scalar work
                    nc.scalar.dma_start(
                        out=wpk3[:, :, 2 * C : WPB], in_=wpack3[:, :, 2 * C : WPB]
                    )
                    nc.scalar.dma_start(out=wof[:], in_=wof_d[:])
                if i >= NSUPER - 2:
                    psQ = pkv.tile([128, 1024], F32, name="psKV", tag="psKV")
                    q_mms_into(i - (NSUPER - 2), psQ)
                    eq_act(i - (NSUPER - 2), psQ)
            ctx_mms(NSUPER - 1)
            kvsb.release()
            pkv.release()

            # ===== epilogue part 1: normalize context rows.  Only the tiny
            # [128,1] reciprocals ride DVE; the scaling copy goes to ACT
            # (per-partition scale vector) so the DVE can start the Q-phase
            # recip/qmul chain — the critical serial path — immediately.
            rcol = [cst.tile([128, 1], F32, name=f"rcol{u}") for u in range(2)]
            ctx_sb = [cst.tile([128, C], SDT, name=f"ctx{u}") for u in range(2)]
            for u in range(2):
                nc.vector.reciprocal(rcol[u][:], psCtx[u][:, C : C + 1])
            for u in range(2):
                nc.scalar.activation(
                    out=ctx_sb[u][:], in_=psCtx[u][:, 0:C],
                    func=AF.Copy, scale=rcol[u][:],
                )
            pctx.release()

            # pool stack is LIFO: psq/psd/po/pw pushed after pctx popped.
            # (Program order doesn't serialize engines — the PE still runs
            # dfull(0) right after ctx(15); only data deps matter.)
            psqp = tc.alloc_tile_pool(name="psq", bufs=1, space="PSUM")
            psdp = tc.alloc_tile_pool(name="psd", bufs=2, space="PSUM")
            pw = tc.alloc_tile_pool(name="psum_w", bufs=1, space="PSUM")

            # chunks 0-1's softmax chains run across the seam, shortening the
            # post-KV DVE serial chain (the Q phase's critical path).  The
            # W-fold matmuls sit between them so the PE never waits on the
            # psd WAR (dfull(1) needs recip(0) to have drained psD).
            dfull_mms(0)
            recip_qmul(0)

            # ===== epilogue part 2: fold wo, W^T = ctx.T @ woT =====
            psW = [pw.tile([128, C], F32, name=f"psW{v}") for v in range(2)]
            for v in range(2):
                for u in range(2):
                    nc.tensor.matmul(
                        psW[v][:],
                        ctx_sb[u][:, ts(v, 128)],
                        woT_sb[u],
                        start=(u == 0),
                        stop=(u == 1) and not use_bv,
                        skip_group_check=True,
                    )
                if use_bv:
                    # context gains +bv[d'] per row (sum_n k = 1), so
                    # W^T += bv (X) rowsum(wo): a K=1 rank-1 matmul.
                    nc.tensor.matmul(
                        psW[v][:],
                        bv_sb[:, ts(v, 128)],
                        wosum_sb[:],
                        start=False,
                        stop=True,
                        skip_group_check=True,
                    )
                # WT copy on ACT — the DVE is saturated at the seam with the
                # hoisted recip/qmul chains
                nc.scalar.activation(out=WT_sb[v][:], in_=psW[v][:], func=AF.Copy)
            dfull_mms(1)
            recip_qmul(1)

            def out_mms(c, psO):
                qt = qts.pop(c)
                for t in range(2):
                    for u in range(2):
                        nc.tensor.matmul(
                            psO[:, t * 512 : (t + 1) * 512],
                            WT_sb[u][:, ts(t, 128)],
                            qt[:, u * 512 : (u + 1) * 512],
                            start=(u == 0),
                            stop=(u == 1),
                        )

            def out_copy_store(c, psO):
                o2 = o2p.tile([128, 2, 512], ODT, name="o2", tag="o2")
                o2f = o2[:].rearrange("p t n -> p (t n)")
                if use_bo:
                    for t in range(2):
                        nc.vector.tensor_scalar_add(
                            out=o2[:, t, :],
                            in0=psO[:, t * 512 : (t + 1) * 512],
                            scalar1=bo_sb[t][:],
                        )
                    nc.sync.dma_start(out=y2[:, :, ts(c, 512)], in_=o2[:])
                    return
                # o2's flat free layout matches psO's supertile order, so ONE
                # full-width ACT copy does the whole cast (ACT has the slack;
                # DVE is the critical engine in the Q loop).  The last chunk
                # instead splits copy ACT/DVE (DVE is done by then) and its
                # halves ride two DMA queues, so the final store drains fast.
                if c == NCHUNKS - 1:
                    nc.scalar.activation(
                        out=o2[:, 0, :], in_=psO[:, 0:512], func=AF.Copy
                    )
                    nc.sync.dma_start(out=y2[:, 0:1, ts(c, 512)], in_=o2[:, 0:1, :])
                    nc.vector.tensor_copy(o2[:, 1, :], psO[:, 512:1024])
                    nc.scalar.dma_start(out=y2[:, 1:2, ts(c, 512)], in_=o2[:, 1:2, :])
                else:
                    nc.scalar.activation(out=o2f, in_=psO[:], func=AF.Copy)
                    nc.sync.dma_start(out=y2[:, :, ts(c, 512)], in_=o2[:])

            # ===== Q main loop: stages q(j), eq(j), Dfull(j-2), recip/qmul
            # (j-2), out(j-3), copy+store(j-3).  Chunks 0-1's q/eq ran in the
            # KV tail; chunk 0's Dfull/recip/qmul ran in the epilogue. =====
            po = None
            psOs = {}
            for j in range(2, NCHUNKS + 3):
                if j >= NCHUNKS:
                    # the drain has no q supertiles; filler keeps HAM warm so
                    # the last out matmuls run at 2.4 GHz, not 1.2
                    warm_ldw(6)
                if 2 <= j - 2 <= NCHUNKS - 1:
                    dfull_mms(j - 2)
                if j - 3 >= 0:
                    c = j - 3
                    psO = po.tile([128, 1024], F32, name="psO", tag="psO")
                    out_mms(c, psO)
                    psOs[c] = psO
                if j <= NCHUNKS - 1:
                    psQ = psqp.tile([128, 1024], F32, name="psQ", tag="psQ")
                    q_mms_into(j, psQ)
                    eq_act(j, psQ)
                if 2 <= j - 2 <= NCHUNKS - 1:
                    recip_qmul(j - 2)
                if j - 3 >= 0:
                    out_copy_store(j - 3, psOs.pop(j - 3))
                if j == 2:
                    # pw's banks free after the WT copies -> the out pool
                    pw.release()
                    po = tc.alloc_tile_pool(name="pso", bufs=1, space="PSUM")
            po.release()
            psdp.release()
            psqp.release()

    nc.compile()
    return nc


def _get_nc(use_bq, use_bo, use_bv, mm_dtype):
    key = (use_bq, use_bo, use_bv, str(mm_dtype))
    if key not in _CACHE:
        with _single_act_table():
            _CACHE[key] = _build(use_bq, use_bo, use_bv, mm_dtype)
    return _CACHE[key]


def _to_mdt(a, mm_dtype):
    if mm_dtype == BF16:
        import ml_dtypes

        return np.ascontiguousarray(a.astype(ml_dtypes.bfloat16))
    return np.ascontiguousarray(a)


def kernel(x, cproj, wq, bq, wkv, bkv, wo, bo, _mm_dtype=BF16, _results_hook=None):
    x = np.ascontiguousarray(np.asarray(x, dtype=np.float32).reshape(B, C, N))
    cf = np.ascontiguousarray(np.asarray(cproj, dtype=np.float32).reshape(B, C, N))
    wq = np.asarray(wq, dtype=np.float32)
    wkv = np.asarray(wkv, dtype=np.float32)
    wo = np.asarray(wo, dtype=np.float32)
    bq = np.asarray(bq, dtype=np.float32)
    bkv = np.asarray(bkv, dtype=np.float32)
    bo = np.asarray(bo, dtype=np.float32)

    use_bq = bool(np.any(bq != 0))
    use_bo = bool(np.any(bo != 0))
    bv = bkv[C:]
    use_bv = bool(np.any(bv != 0))

    wqT = np.ascontiguousarray(wq.T)
    wkvT = np.ascontiguousarray(wkv.T)
    woT = np.ascontiguousarray(wo.T)

    # packed weights: bf16 [wkvT | wqT] per c-half, f32 [woT] per c-half
    wpack = np.zeros((128, 2 * WPB), np.float32)
    wof = np.zeros((128, 2 * C), np.float32)
    for u in range(2):
        r = slice(u * 128, (u + 1) * 128)
        wpack[:, u * WPB : u * WPB + 2 * C] = wkvT[r]
        wpack[:, u * WPB + 2 * C : u * WPB + 3 * C] = wqT[r]
        wof[:, u * C : (u + 1) * C] = woT[r]

    nc = _get_nc(use_bq, use_bo, use_bv, _mm_dtype)

    base = {
        "wpack": _to_mdt(wpack, _mm_dtype),
        "wof": wof,
    }
    if use_bq:
        base["bq_s"] = (SCALE * bq).reshape(C, 1)
    if use_bo:
        base["bo_c"] = bo.reshape(C, 1)
    if use_bv:
        base["bv_r"] = bv.reshape(1, C)
        base["wosum"] = wo.sum(axis=1).reshape(1, C)

    in_maps = [
        dict(base, x=_to_mdt(x[b], _mm_dtype), cp=_to_mdt(cf[b], _mm_dtype))
        for b in range(B)
    ]
    res = run_bass_kernel_spmd(nc, in_maps, list(range(NCORES)))
    if _results_hook is not None:
        _results_hook(res)
    out = np.stack(
        [np.asarray(res.results[b]["y"], dtype=np.float32) for b in range(B)],
        axis=0,
    )
    return out.reshape(B, C, H, W)


# revision 33
# speedup vs baseline: 1.2176x; 1.0437x over previous
"""Trainium2 Bass kernel for CrossEfficientAttention (B=8, C=256, H=W=64, 4 heads).

Sharding: data-parallel over batch B — one sample per NeuronCore, no collectives.

Per-core math (sample x_s, c_s of shape [C, N], N = H*W = 4096):
    Q  = wq @ x_s                      (+ bq, folded into the exp's ACT bias)
    KV = wkv @ c_s                     (bkv[:C] cancels exactly in softmax over N;
                                        bkv[C:] handled as a rank-1 update of W)
    k  = softmax_N(K); q = softmax_head(Q * C**-0.25)
    context = k @ V^T ; out = wo @ (context @ q) + bo

Restructured for the PE array (out = lhsT.T @ rhs, contraction over partitions):
  * KV^T computed directly in [N, C] layout by using c_s tiles as lhsT.
  * k-softmax normalizer: ones-columns appended to V^T give row sums of exp(K)
    in column 256 of the context PSUM accumulator; context rows are then scaled
    by the reciprocal column (per-partition tensor_scalar) — no transposes.
  * wo folded in early: W^T = matmul(lhsT=context, rhs=wo^T) directly in [d, o]
    layout. The per-chunk output is then just out2 = W^T.T @ q.
  * q-softmax denominators via a block-diagonal ones matrix ("Amat",
    blockdiag(J64, J64), identical for both channel halves): Dfull = Amat.T @ eq
    puts D[head(c), n] at every partition c directly — no 4-row D tile, no
    Ln/Exp round-trip, no selector-broadcast matmul.  1/D via the single-
    instruction DVE reciprocal_approx_fast (~51 ULP), and q = eq * rD with the
    multiply split DVE/GpSimd so no single engine becomes the bottleneck.
  * Q phase paced evenly: at iteration j the PE runs Dfull(j-2), out(j-3),
    q(j) back to back; ACT does exp(j) + half of copy(j-3); DVE does
    recip(j-2), half of qmul(j-2), half of copy(j-3); GpSimd does the other
    qmul half.  No long serialized drain — the last chunk's store follows its
    matmuls by ~1.5us, and its two halves ride two DMA queues.

Hard-won scheduling facts (measured on hardware):
  * The PE HAM clock gate passes 4/8 pulses (1.2 GHz) until it has seen a full
    ~3.4us busy window, then 8/8 (2.4 GHz).  The framework preamble takes
    ~6.6us and the first input chunk lands ~2.5us after its trigger, so with
    no warmup the first ~6us of real matmuls run at half clock.  Warmup
    matmuls on a zeroed SBUF tile, sized to end just as the first data lands,
    start the busy window early so real work runs warm almost immediately.
  * DMA trigger instructions cost ~0.65us of issuing-engine queue time each,
    and the first bytes of a queue move ~1.6us after the trigger.  Packet
    count per transfer is rows/16-queues; a 128-col one-c-half slice of cf is
    8 packets/queue and lands ~2us after its trigger.
  * Inputs (x, cp, wkv, wq) and the y output ride in bf16 — halves both
    the input stream and the store stream for ~4.1e-3 rel err; all on-chip
    intermediates stay f32r (same PE speed). fp8 measured 4.6e-2 rel err
    (per-element quantization noise does not average out in the cancelling
    context/out sums), well over the 2e-2 gate.
  * An SBUF->SBUF broadcast DMA (stride-0 source) crashed the device
    (NRT_EXEC_UNIT_UNRECOVERABLE) — broadcasts go through the PE.
"""

import contextlib

import numpy as np

import concourse.bass as bass
import concourse.tile as tile
from concourse import bacc, mybir
from concourse.bass import ts
from concourse.bass_utils import run_bass_kernel_spmd

B, C, H, W = 8, 256, 64, 64
N = H * W
NHEADS = 4
DHEAD = C // NHEADS
NCORES = 8
NSUPER = N // 256          # 16 double-n-tile iterations for the KV phase
NCHUNKS = N // 512         # 8 column chunks for the Q/output phase
SCALE = float(1.0 / np.sqrt(np.sqrt(np.float32(C))))
VW = C + 2                 # V^T tile row width (256 data + 2 ones cols; fp32r needs even free)
WPB = 2 * C + C            # bf16 packed row width per c-half: wkvT|wqT
NWARM = 5                  # warmup matmuls (N=512 cold ~= 0.61us each).  The
                           # first gating DMAs land anywhere in 11-15us run to
                           # run, so dummy-LDWEIGHTS filler (107ns each, no
                           # PSUM writes, safe between matmul groups because
                           # matmuls self-load their weights) bridges the
                           # variance and keeps the HAM busy-window filling.

F32 = mybir.dt.float32
F32R = mybir.dt.float32r
BF16 = mybir.dt.bfloat16
AF = mybir.ActivationFunctionType

_CACHE = {}


def _single_act_table():
    """Scope-patch the activation-table list so the table-load pass resolves
    Exp (and Ln, unused now) to natural_log_exp_and_others (set ids stay
    positional, so only the function lists may change, not the order)."""
    import concourse.bacc as cbacc
    from concourse.hw_specs import get_activation_tables

    @contextlib.contextmanager
    def scope():
        orig = cbacc.get_activation_tables

        def patched(arch):
            tabs = get_activation_tables(arch)
            return {
                k: (v if k == "natural_log_exp_and_others" else set())
                for k, v in tabs.items()
            }

        cbacc.get_activation_tables = patched
        try:
            yield
        finally:
            cbacc.get_activation_tables = orig

    return scope()


def _build(use_bq, use_bo, use_bv, mm_dtype):
    nc = bacc.Bacc("TRN2", target_bir_lowering=False, debug=False)
    # IDT: dtype of the DMA-heavy inputs (x, cp, wkv, wq) — bf16 halves the
    # input stream with one rounding ahead of the softmax averaging.
    # SDT: on-chip intermediates stay f32r.
    IDT = mm_dtype
    SDT = F32R
    # KV-context operands (exp(K), V^T) in bf16: the 64 ctx matmuls drop from
    # ~125ns to ~110ns each and the KV phase ends ~1us earlier (everything
    # downstream is serialized behind it).  ~+0.5% rel err via the cancelling
    # context sum — budget is 2e-2, measured ~4e-3 before this.
    KDT = BF16 if mm_dtype == BF16 else F32R

    x = nc.dram_tensor("x", [C, N], IDT, kind="ExternalInput")
    cp = nc.dram_tensor("cp", [C, N], IDT, kind="ExternalInput")
    wpack = nc.dram_tensor("wpack", [128, 2 * WPB], IDT, kind="ExternalInput")
    wof_d = nc.dram_tensor("wof", [128, 2 * C], F32R, kind="ExternalInput")
    if use_bq:
        bq_s = nc.dram_tensor("bq_s", [C, 1], F32, kind="ExternalInput")
    if use_bo:
        bo_c = nc.dram_tensor("bo_c", [C, 1], F32, kind="ExternalInput")
    if use_bv:
        bv_r = nc.dram_tensor("bv_r", [1, C], F32R, kind="ExternalInput")
        wosum = nc.dram_tensor("wosum", [1, C], F32R, kind="ExternalInput")
    ODT = BF16 if mm_dtype == BF16 else F32
    y = nc.dram_tensor("y", [C, N], ODT, kind="ExternalOutput")

    # DRAM views with the c-half dim split out so one DMA covers both halves
    cp2 = cp[:].rearrange("(u p) n -> p u n", u=2)
    x2 = x[:].rearrange("(u p) n -> p u n", u=2)
    y2 = y[:].rearrange("(u p) n -> p u n", u=2)

    with tile.TileContext(nc) as tc:
        with (
            tc.tile_pool(name="const", bufs=1) as cst,
            tc.tile_pool(name="big", bufs=1) as big,
            tc.tile_pool(name="eqp", bufs=4) as eqp,
            tc.tile_pool(name="rdp", bufs=3) as rdp,
            tc.tile_pool(name="qtp", bufs=3) as qtp,
            tc.tile_pool(name="o2p", bufs=4) as o2p,
        ):
            # --- warmup fodder: zeroed tile, no input dependencies
            # (bf16: ldweights() rejects f32/f32r) ---
            zt = cst.tile([128, 512], BF16, name="zt")
            nc.vector.memset(zt[:].bitcast(F32), 0.0)

            def warm_ldw(n):
                # dummy weight loads: PE-busy filler with no PSUM side
                # effects; each delays real work by at most ~107ns
                for _ in range(n):
                    nc.tensor.ldweights(zt[:, 0:128])

            # --- head-indicator block-diagonal matrix (identical for both
            # c-halves: heads are 64 channels, aligned within each 128-half)
            amat = cst.tile([128, 128], SDT, name="amat")
            nc.vector.memset(amat[:].bitcast(F32), 0.0)
            nc.vector.memset(amat[0:64, 0:64].bitcast(F32), 1.0)
            nc.vector.memset(amat[64:128, 64:128].bitcast(F32), 1.0)

            # manually-rotated V^T ring: ones columns pre-set once.  memset
            # writes a 4-byte pattern: f32 1.0, or a packed pair of bf16 1.0s.
            NVBUF = 4
            ones_pat = (
                float(np.frombuffer(np.uint32(0x3F803F80).tobytes(), np.float32)[0])
                if KDT == BF16
                else 1.0
            )
            v2r = [cst.tile([128, 2 * VW], KDT, name=f"v2_{i}") for i in range(NVBUF)]
            for i in range(NVBUF):
                for h in range(2):
                    o = v2r[i][:, h * VW + C : h * VW + C + 2].bitcast(F32)
                    nc.vector.memset(o, ones_pat)

            # --- packed weights; the KV-phase slice (wkvT) rides first ---
            wpk = cst.tile([128, 2 * WPB], IDT, name="wpk")
            wpk3 = wpk[:].rearrange("p (u w) -> p u w", u=2)
            wpack3 = wpack[:].rearrange("p (u w) -> p u w", u=2)
            wof = cst.tile([128, 2 * C], SDT, name="wof")
            wkvT_sb = [wpk[:, u * WPB : u * WPB + 2 * C] for u in range(2)]
            wqT_sb = [wpk[:, u * WPB + 2 * C : u * WPB + 3 * C] for u in range(2)]
            woT_sb = [wof[:, u * C : (u + 1) * C] for u in range(2)]

            cf_sb = big.tile([128, 2, N], IDT, name="cf_sb")
            xf_sb = big.tile([128, 2, N], IDT, name="xf_sb")

            # --- input triggers: early DMA runs at only ~130 GB/s aggregate,
            # so the first-iteration gating set (wkvT both halves + first cf
            # slices) is split ACROSS the two HWDGE queues (sync + scalar) so
            # the halves land in parallel, and the cf gates are 128-col
            # per-c-half slices (32 KB).  Only SP and ACT have HWDGE queues.
            nc.sync.dma_start(out=wpk3[:, 0, 0 : 2 * C], in_=wpack3[:, 0, 0 : 2 * C])
            nc.scalar.dma_start(out=wpk3[:, 1, 0 : 2 * C], in_=wpack3[:, 1, 0 : 2 * C])
            nc.sync.dma_start(out=cf_sb[:, 0:1, 0:128], in_=cp2[:, 0:1, 0:128])
            nc.scalar.dma_start(out=cf_sb[:, 1:2, 0:128], in_=cp2[:, 1:2, 0:128])
            nc.sync.dma_start(out=cf_sb[:, 0:1, 128:256], in_=cp2[:, 0:1, 128:256])
            nc.scalar.dma_start(out=cf_sb[:, 1:2, 128:256], in_=cp2[:, 1:2, 128:256])
            nc.scalar.dma_start(out=cf_sb[:, :, 256:512], in_=cp2[:, :, 256:512])
            # bulk: cf then x, graduated, all on sync (scalar goes back to ACT
            # work; wqT/wof triggers are issued mid-KV-loop below)
            nc.sync.dma_start(out=cf_sb[:, :, 512:1024], in_=cp2[:, :, 512:1024])
            nc.sync.dma_start(out=cf_sb[:, :, 1024:2048], in_=cp2[:, :, 1024:2048])
            nc.sync.dma_start(out=cf_sb[:, :, 2048:4096], in_=cp2[:, :, 2048:4096])
            nc.sync.dma_start(out=xf_sb[:, :, 0:2048], in_=x2[:, :, 0:2048])
            nc.sync.dma_start(out=xf_sb[:, :, 2048:4096], in_=x2[:, :, 2048:4096])
            if use_bq:
                bq_sb = [cst.tile([128, 1], F32, name=f"bq{u}") for u in range(2)]
                for u in range(2):
                    nc.scalar.dma_start(out=bq_sb[u][:], in_=bq_s[ts(u, 128), :])
            if use_bo:
                bo_sb = [cst.tile([128, 1], F32, name=f"bo{u}") for u in range(2)]
                for u in range(2):
                    nc.scalar.dma_start(out=bo_sb[u][:], in_=bo_c[ts(u, 128), :])
            if use_bv:
                bv_sb = cst.tile([1, C], SDT, name="bv_sb")
                nc.scalar.dma_start(out=bv_sb[:], in_=bv_r[:])
                wosum_sb = cst.tile([1, C], SDT, name="wosum_sb")
                nc.scalar.dma_start(out=wosum_sb[:], in_=wosum[:])

            # persistent W^T tiles (filled in the epilogue)
            WT_sb = [cst.tile([128, C], SDT, name=f"WT{u}") for u in range(2)]

            eqs, psDs, qts = {}, {}, {}

            def q_mms_into(j, psQ):
                for t in range(2):
                    for u in range(2):
                        nc.tensor.matmul(
                            psQ[:, t * 512 : (t + 1) * 512],
                            wqT_sb[u][:, ts(t, 128)],
                            xf_sb[:, u, ts(j, 512)],
                            start=(u == 0),
                            stop=(u == 1),
                        )

            def eq_act(j, psQ):
                eq = eqp.tile([128, 1024], SDT, name="eq", tag="eq")
                if use_bq:
                    for t in range(2):
                        nc.scalar.activation(
                            out=eq[:, t * 512 : (t + 1) * 512],
                            in_=psQ[:, t * 512 : (t + 1) * 512],
                            func=AF.Exp,
                            scale=SCALE,
                            bias=bq_sb[t][:],
                        )
                else:
                    nc.scalar.activation(
                        out=eq[:], in_=psQ[:], func=AF.Exp, scale=SCALE
                    )
                eqs[j] = eq

            def dfull_mms(j, pool=None, tag="psD"):
                psD = (pool or psdp).tile([128, 1024], F32, name="psD", tag=tag)
                eq = eqs[j]
                for t in range(2):
                    nc.tensor.matmul(
                        psD[:, t * 512 : (t + 1) * 512],
                        amat[:],
                        eq[:, t * 512 : (t + 1) * 512],
                        start=True,
                        stop=True,
                    )
                psDs[j] = psD

            def recip_qmul(j):
                # qt = eq * (1/D).  The TT-ALU has no divide (s3s3d3 ISA
                # check), so it's the single-instruction approx reciprocal
                # plus a multiply.  GpSimd offload measured 2x slower AND it
                # shares the DVE SBUF port — everything rides DVE.
                psD = psDs.pop(j)
                rD = rdp.tile([128, 1024], F32, name="rD", tag="rD")
                nc.vector.reciprocal_approx_fast(out=rD[:], in_=psD[:])
                qt = qtp.tile([128, 1024], SDT, name="qt", tag="qt")
                eq = eqs.pop(j)
                nc.vector.tensor_mul(qt[:], eq[:], rD[:])
                qts[j] = qt

            # ---- warmup: keep the PE (and the HAM busy-window) running from
            # right after the preamble until the first input data lands ----
            pwarm = tc.alloc_tile_pool(name="pswarm", bufs=1, space="PSUM")
            psw = pwarm.tile([128, 512], F32, name="psw")
            for k in range(NWARM):
                nc.tensor.matmul(
                    psw[:], zt[:, 0:128], zt[:],
                    start=True, stop=True, skip_group_check=True,
                )
            warm_ldw(10)
            pwarm.release()

            # ============ KV phase: context = exp(K) @ [V^T | 1] ============
            # Software-pipelined by one iteration: the PE runs iteration i's
            # KV matmuls and iteration i-1's context matmuls back to back.
            # u is the outer loop so iteration 0's first matmuls gate only on
            # the per-c-half 128-col cf slices.
            pctx = tc.alloc_tile_pool(name="psum_ctx", bufs=1, space="PSUM")
            psCtx = [pctx.tile([128, VW], F32, name=f"psCtx{u}") for u in range(2)]
            pkv = tc.alloc_tile_pool(name="psum_kv", bufs=3, space="PSUM")
            kvsb = tc.alloc_tile_pool(name="kvsb", bufs=3)
            eks = {}

            def kv_mms(i):
                psKV = pkv.tile([128, 1024], F32, name="psKV")
                for u in range(2):
                    for h in range(2):
                        nt = 2 * i + h
                        nc.tensor.matmul(
                            psKV[:, h * 512 : (h + 1) * 512],
                            cf_sb[:, u, ts(nt, 128)],
                            wkvT_sb[u],
                            start=(u == 0),
                            stop=(u == 1),
                            skip_group_check=True,
                        )
                return psKV

            def ctx_mms(i):
                ek = eks.pop(i)
                v2 = v2r[i % NVBUF]
                for h in range(2):
                    for u in range(2):
                        nc.tensor.matmul(
                            psCtx[u][:],
                            ek[:, h, ts(u, 128)],
                            v2[:, h * VW : (h + 1) * VW],
                            start=(i == 0 and h == 0),
                            stop=(i == NSUPER - 1 and h == 1),
                            skip_group_check=True,
                        )

            def kv_post(i, psKV):
                ek = kvsb.tile([128, 2, C], KDT, name="ek")
                nc.scalar.activation(
                    out=ek[:],
                    in_=psKV[:].rearrange("p (h c) -> p h c", h=2)[:, :, 0:C],
                    func=AF.Exp,
                )
                eks[i] = ek
                v2 = v2r[i % NVBUF]
                nc.vector.tensor_copy(
                    v2[:].rearrange("p (h w) -> p h w", h=2)[:, :, 0:C],
                    psKV[:].rearrange("p (h c) -> p h c", h=2)[:, :, C : 2 * C],
                )

            # the first two Q chunks ride inside the KV tail (their PSUM
            # supertiles borrow the KV pool's slots) so their eq tiles are
            # ready the moment the KV phase ends
            for i in range(NSUPER):
                psKV = kv_mms(i)
                if i > 0:
                    ctx_mms(i - 1)
                kv_post(i, psKV)
                if i <= 3:
                    # early iterations are DMA-dribble-paced; keep the HAM
                    # busy-window filling across the data waits
                    warm_ldw(6)
                if i == 2:
                    # wqT (needed from iter 14) + woT (epilogue): issue these
                    # scalar-queue triggers after the first ek ACTs so they
                    # don't delay the KV pipeline's scalar work
                    nc.scalar.dma_start(
                        out=wpk3[:, :, 2 * C : WPB], in_=wpack3[:, :, 2 * C : WPB]
                    )
                    nc.scalar.dma_start(out=wof[:], in_=wof_d[:])
                if i >= NSUPER - 2:
                    psQ = pkv.tile([128, 1024], F32, name="psKV", tag="psKV")
                    q_mms_into(i - (NSUPER - 2), psQ)
                    eq_act(i - (NSUPER - 2), psQ)
            ctx_mms(NSUPER - 1)
            kvsb.release()
            pkv.release()

            # ===== epilogue part 1: normalize context rows.  Only the tiny
            # [128,1] reciprocals ride DVE; the scaling copy goes to ACT
            # (per-partition scale vector) so the DVE can start the Q-phase
            # recip/qmul chain — the critical serial path — immediately.
            rcol = [cst.tile([128, 1], F32, name=f"rcol{u}") for u in range(2)]
            ctx_sb = [cst.tile([128, C], SDT, name=f"ctx{u}") for u in range(2)]
            for u in range(2):
                nc.vector.reciprocal(rcol[u][:], psCtx[u][:, C : C + 1])
            for u in range(2):
                nc.scalar.activation(
                    out=ctx_sb[u][:], in_=psCtx[u][:, 0:C],
                    func=AF.Copy, scale=rcol[u][:],
                )
            pctx.release()

            # pool stack is LIFO: psq/psd/po/pw pushed after pctx popped.
            # (Program order doesn't serialize engines — the PE still runs
            # dfull(0) right after ctx(15); only data deps matter.)
            psqp = tc.alloc_tile_pool(name="psq", bufs=1, space="PSUM")
            psdp = tc.alloc_tile_pool(name="psd", bufs=2, space="PSUM")
            pw = tc.alloc_tile_pool(name="psum_w", bufs=1, space="PSUM")

            # chunks 0-1's softmax chains run across the seam, shortening the
            # post-KV DVE serial chain (the Q phase's critical path).  The
            # W-fold matmuls sit between them so the PE never waits on the
            # psd WAR (dfull(1) needs recip(0) to have drained psD).
            dfull_mms(0)
            recip_qmul(0)

            # ===== epilogue part 2: fold wo, W^T = ctx.T @ woT =====
            psW = [pw.tile([128, C], F32, name=f"psW{v}") for v in range(2)]
            for v in range(2):
                for u in range(2):
                    nc.tensor.matmul(
                        psW[v][:],
                        ctx_sb[u][:, ts(v, 128)],
                        woT_sb[u],
                        start=(u == 0),
                        stop=(u == 1) and not use_bv,
                        skip_group_check=True,
                    )
                if use_bv:
                    # context gains +bv[d'] per row (sum_n k = 1), so
                    # W^T += bv (X) rowsum(wo): a K=1 rank-1 matmul.
                    nc.tensor.matmul(
                        psW[v][:],
                        bv_sb[:, ts(v, 128)],
                        wosum_sb[:],
                        start=False,
                        stop=True,
                        skip_group_check=True,
                    )
                # WT copy on ACT — the DVE is saturated at the seam with the
                # hoisted recip/qmul chains
                nc.scalar.activation(out=WT_sb[v][:], in_=psW[v][:], func=AF.Copy)
            dfull_mms(1)
            recip_qmul(1)

            def out_mms(c, psO):
                qt = qts.pop(c)
                for t in range(2):
                    for u in range(2):
                        nc.tensor.matmul(
                            psO[:, t * 512 : (t + 1) * 512],
                            WT_sb[u][:, ts(t, 128)],
                            qt[:, u * 512 : (u + 1) * 512],
                            start=(u == 0),
                            stop=(u == 1),
                        )

            def out_copy_store(c, psO):
                o2 = o2p.tile([128, 2, 512], ODT, name="o2", tag="o2")
                o2f = o2[:].rearrange("p t n -> p (t n)")
                if use_bo:
                    for t in range(2):
                        nc.vector.tensor_scalar_add(
                            out=o2[:, t, :],
                            in0=psO[:, t * 512 : (t + 1) * 512],
                            scalar1=bo_sb[t][:],
                        )
                    nc.sync.dma_start(out=y2[:, :, ts(c, 512)], in_=o2[:])
                    return
                # o2's flat free layout matches psO's supertile order, so ONE
                # full-width ACT copy does the whole cast (ACT has the slack;
                # DVE is the critical engine in the Q loop).  The last chunk
                # instead splits copy ACT/DVE (DVE is done by then) and its
                # halves ride two DMA queues, so the final store drains fast.
                if c == NCHUNKS - 1:
                    nc.scalar.activation(
                        out=o2[:, 0, :], in_=psO[:, 0:512], func=AF.Copy
                    )
                    nc.sync.dma_start(out=y2[:, 0:1, ts(c, 512)], in_=o2[:, 0:1, :])
                    nc.vector.tensor_copy(o2[:, 1, :], psO[:, 512:1024])
                    nc.scalar.dma_start(out=y2[:, 1:2, ts(c, 512)], in_=o2[:, 1:2, :])
                else:
                    nc.scalar.activation(out=o2f, in_=psO[:], func=AF.Copy)
                    nc.sync.dma_start(out=y2[:, :, ts(c, 512)], in_=o2[:])

            # ===== Q main loop: stages q(j), eq(j), Dfull(j-2), recip/qmul
            # (j-2), out(j-3), copy+store(j-3).  Chunks 0-1's q/eq ran in the
            # KV tail; chunk 0's Dfull/recip/qmul ran in the epilogue. =====
            po = None
            psOs = {}
            for j in range(2, NCHUNKS + 3):
                if j >= NCHUNKS:
                    # the drain has no q supertiles; filler keeps HAM warm so
                    # the last out matmuls run at 2.4 GHz, not 1.2
                    warm_ldw(12)
                if 2 <= j - 2 <= NCHUNKS - 1:
                    dfull_mms(j - 2)
                if j - 3 >= 0:
                    c = j - 3
                    if c == NCHUNKS - 2:
                        # psq is dead after eq(7); out(6) borrows its banks so
                        # the drain's back-to-back outs don't serialize on the
                        # single out pool's PSUM->SBUF copies
                        psO = psqp.tile([128, 1024], F32, name="psO6", tag="psQ")
                    else:
                        psO = po.tile([128, 1024], F32, name="psO", tag="psO")
                    out_mms(c, psO)
                    psOs[c] = psO
                if j <= NCHUNKS - 1:
                    psQ = psqp.tile([128, 1024], F32, name="psQ", tag="psQ")
                    q_mms_into(j, psQ)
                    eq_act(j, psQ)
                if 2 <= j - 2 <= NCHUNKS - 1:
                    recip_qmul(j - 2)
                if j - 3 >= 0:
                    out_copy_store(j - 3, psOs.pop(j - 3))
                if j == 2:
                    # pw's banks free after the WT copies -> the out pool
                    pw.release()
                    po = tc.alloc_tile_pool(name="pso", bufs=1, space="PSUM")
            po.release()
            psdp.release()
            psqp.release()

    nc.compile()
    return nc


def _get_nc(use_bq, use_bo, use_bv, mm_dtype):
    key = (use_bq, use_bo, use_bv, str(mm_dtype))
    if key not in _CACHE:
        with _single_act_table():
            _CACHE[key] = _build(use_bq, use_bo, use_bv, mm_dtype)
    return _CACHE[key]


def _to_mdt(a, mm_dtype):
    if mm_dtype == BF16:
        import ml_dtypes

        return np.ascontiguousarray(a.astype(ml_dtypes.bfloat16))
    return np.ascontiguousarray(a)


def kernel(x, cproj, wq, bq, wkv, bkv, wo, bo, _mm_dtype=BF16, _results_hook=None):
    x = np.ascontiguousarray(np.asarray(x, dtype=np.float32).reshape(B, C, N))
    cf = np.ascontiguousarray(np.asarray(cproj, dtype=np.float32).reshape(B, C, N))
    wq = np.asarray(wq, dtype=np.float32)
    wkv = np.asarray(wkv, dtype=np.float32)
    wo = np.asarray(wo, dtype=np.float32)
    bq = np.asarray(bq, dtype=np.float32)
    bkv = np.asarray(bkv, dtype=np.float32)
    bo = np.asarray(bo, dtype=np.float32)

    use_bq = bool(np.any(bq != 0))
    use_bo = bool(np.any(bo != 0))
    bv = bkv[C:]
    use_bv = bool(np.any(bv != 0))

    wqT = np.ascontiguousarray(wq.T)
    wkvT = np.ascontiguousarray(wkv.T)
    woT = np.ascontiguousarray(wo.T)

    # packed weights: bf16 [wkvT | wqT] per c-half, f32 [woT] per c-half
    wpack = np.zeros((128, 2 * WPB), np.float32)
    wof = np.zeros((128, 2 * C), np.float32)
    for u in range(2):
        r = slice(u * 128, (u + 1) * 128)
        wpack[:, u * WPB : u * WPB + 2 * C] = wkvT[r]
        wpack[:, u * WPB + 2 * C : u * WPB + 3 * C] = wqT[r]
        wof[:, u * C : (u + 1) * C] = woT[r]

    nc = _get_nc(use_bq, use_bo, use_bv, _mm_dtype)

    base = {
        "wpack": _to_mdt(wpack, _mm_dtype),
        "wof": wof,
    }
    if use_bq:
        base["bq_s"] = (SCALE * bq).reshape(C, 1)
    if use_bo:
        base["bo_c"] = bo.reshape(C, 1)
    if use_bv:
        base["bv_r"] = bv.reshape(1, C)
        base["wosum"] = wo.sum(axis=1).reshape(1, C)

    in_maps = [
        dict(base, x=_to_mdt(x[b], _mm_dtype), cp=_to_mdt(cf[b], _mm_dtype))
        for b in range(B)
    ]
    res = run_bass_kernel_spmd(nc, in_maps, list(range(NCORES)))
    if _results_hook is not None:
        _results_hook(res)
    out = np.stack(
        [np.asarray(res.results[b]["y"], dtype=np.float32) for b in range(B)],
        axis=0,
    )
    return out.reshape(B, C, H, W)
